# revision 1
# baseline (speedup 1.0000x reference)
"""Trainium2 Bass kernel for a context-LSTM decoder.

Model (B=256, T=256, I=H=1024, 4H=4096, F=512, NC=7):
    ctx   = v @ Wc.T + (bc + bi + bh)                      # [B, 4H], const over t
    gx    = i_features @ Wi.T + ctx                        # [B, T, 4H]
    per t: gates = gx[t] + h @ Wh.T ; LSTM cell update
    out   = relu(h_T @ Wfa.T + bfa) @ Wfc.T + bfc          # [B, 7]

Strategy: pure data-parallel over batch, 32 rows per core, no collectives.
All matmul operands fp16 (fp32 PSUM accumulation), cell state c fp32.
Host pre-transposes x to [I, T, Bs] and weights to K-major so every DMA is
contiguous; the only on-device transposes are the per-step h -> h_T
(DMA xbar transpose, off the critical PE path).

Phase 1 (big GEMM) tiles M=(4t x 32b)=128 rows at full PE width; phase 2
streams Wh through the PE as the moving operand (64 matmuls of N=512 per
step), which is the per-step floor; DVE adds gx during PSUM drain and ACT
does sigmoid/tanh (one shared table set).
"""

import numpy as np
from contextlib import ExitStack

import concourse.bass as bass
import concourse.bacc as bacc
import concourse.mybir as mybir
import concourse.tile as tile
from concourse.bass_utils import run_bass_kernel_spmd

B, T, I = 256, 256, 1024
H = 1024
G = 4 * H
F = 512
NCLS = 7
NCORES = 8
BS = B // NCORES          # 32 batch rows per core
P = 128                   # partitions
KI = I // P               # 8 k-tiles over the input dim
KH = H // P               # 8 k-tiles over the hidden dim
NCH = G // 512            # 8 chunks of 512 gate columns
F16 = mybir.dt.float16
F32 = mybir.dt.float32
AOP = mybir.AluOpType
AFT = mybir.ActivationFunctionType

# chunk n holds gate columns [512n, 512n+512); ig=0,1 fg=2,3 cg=4,5 og=6,7.
# Emit in this order so half 0 (chunks 0,2,4,6 -> units 0:512) finishes first.
CHUNK_ORDER = [0, 2, 4, 6, 1, 3, 5, 7]


def build_lstm(t_steps: int = T, rec_steps: int | None = None,
               no_gx_mm: bool = False, gx_via_dve: bool = True,
               ph1_wr_engine: str = "gpsimd", ph1_no_write: bool = False,
               ph1_no_drain: bool = False):
    # rec_steps: number of recurrence iterations (defaults to t_steps). When
    # larger than t_steps, gx rows are reused cyclically (timing studies only).
    if rec_steps is None:
        rec_steps = t_steps
    assert t_steps % 4 == 0
    n_mtiles = t_steps * BS // P          # phase-1 M-tiles (128 bt-rows each)
    nc = bacc.Bacc("TRN2", target_bir_lowering=False, debug=False,
                   num_devices=NCORES)

    xT = nc.declare_dram_parameter("xT", [I, t_steps, BS], F16, isOutput=False)
    vT = nc.declare_dram_parameter("vT", [I, BS], F16, isOutput=False)
    WiT = nc.declare_dram_parameter("WiT", [I, G], F16, isOutput=False)
    WhT = nc.declare_dram_parameter("WhT", [H, G], F16, isOutput=False)
    WcT = nc.declare_dram_parameter("WcT", [I, G], F16, isOutput=False)
    WfaT = nc.declare_dram_parameter("WfaT", [H, F], F16, isOutput=False)
    WfcT = nc.declare_dram_parameter("WfcT", [F, NCLS], F16, isOutput=False)
    bias = nc.declare_dram_parameter("bias_total", [G], F32, isOutput=False)
    bfa = nc.declare_dram_parameter("bfa", [F], F32, isOutput=False)
    bfc = nc.declare_dram_parameter("bfc", [NCLS], F32, isOutput=False)
    ident = nc.declare_dram_parameter("ident32", [BS, BS], F16, isOutput=False)
    out = nc.declare_dram_parameter("out", [BS, NCLS], F32, isOutput=True)

    gx = nc.dram_tensor("gx", [t_steps, BS, G], F16)

    # K-major views of DRAM tensors: i = k*128 + p
    xT_r = xT[:].rearrange("(k p) t b -> p k (t b)", p=P)
    WiT_r = WiT[:].rearrange("(k p) g -> p k g", p=P)
    WhT_r = WhT[:].rearrange("(k p) g -> p k g", p=P)
    WcT_r = WcT[:].rearrange("(k p) g -> p k g", p=P)
    WfaT_r = WfaT[:].rearrange("(k p) f -> p k f", p=P)
    WfcT_r = WfcT[:].rearrange("(q p) n -> p q n", p=P)
    vT_r = vT[:].rearrange("(k p) b -> p k b", p=P)
    gx_rows = gx[:].rearrange("t b g -> (t b) g")

    def bcast(src_ap, rows):
        # read a [cols] DRAM vector into [rows, cols] SBUF (partition bcast)
        return bass.AP(tensor=src_ap.tensor, offset=src_ap.offset,
                       ap=[[0, rows]] + list(src_ap.ap))

    with tile.TileContext(nc) as tc, ExitStack() as ctx:
        consts = ctx.enter_context(tc.tile_pool(name="consts", bufs=1))

        # ---- small resident constants ----
        bfa_rep = consts.tile([BS, F], F32, tag="bfa_rep")
        nc.sync.dma_start(out=bfa_rep, in_=bcast(bfa[:], BS))
        bfc_rep = consts.tile([BS, NCLS], F32, tag="bfc_rep")
        nc.sync.dma_start(out=bfc_rep, in_=bcast(bfc[:], BS))
        ident_sb = consts.tile([BS, BS], F16, tag="ident_sb")
        nc.sync.dma_start(out=ident_sb, in_=ident[:])

        with (
            tc.tile_pool(name="ph1_big", bufs=1) as ph1_big,
            tc.tile_pool(name="ph1_ps", bufs=4, space="PSUM") as ph1_ps,
            tc.tile_pool(name="ph1_x", bufs=2) as ph1_x,
            tc.tile_pool(name="ph1_out", bufs=3) as ph1_out,
        ):
            ctx4 = ph1_big.tile([P, G], F32, tag="ctx4")
            # ---------- phase 0: ctx4 = v @ WcT + biases, replicated 4x ----------
            with nc.named_scope("phase0_ctx"):
                with (
                    tc.tile_pool(name="ph0_w", bufs=2) as ph0_w,
                    tc.tile_pool(name="ph0_misc", bufs=1) as ph0_misc,
                ):
                    v_sb = ph0_misc.tile([P, KI, BS], F16, tag="v_sb")
                    nc.sync.dma_start(out=v_sb, in_=vT_r)
                    bias_rep = ph0_misc.tile([BS, G], F32, tag="bias_rep")
                    nc.sync.dma_start(out=bias_rep, in_=bcast(bias[:], BS))
                    for n in range(NCH):
                        wc_t = ph0_w.tile([P, KI, 512], F16, tag="wc")
                        nc.sync.dma_start(
                            out=wc_t, in_=WcT_r[:, :, 512 * n:512 * (n + 1)])
                        ps0 = ph1_ps.tile([BS, 512], F32, tag="ps0", bufs=2)
                        for k in range(KI):
                            nc.tensor.matmul(ps0, lhsT=v_sb[:, k, :],
                                             rhs=wc_t[:, k, :],
                                             start=(k == 0), stop=(k == KI - 1))
                        nc.vector.scalar_tensor_tensor(
                            out=ctx4[0:BS, 512 * n:512 * (n + 1)], in0=ps0,
                            scalar=1.0,
                            in1=bias_rep[:, 512 * n:512 * (n + 1)],
                            op0=AOP.mult, op1=AOP.add)
                    for r in range(1, 4):
                        nc.sync.dma_start(out=ctx4[BS * r:BS * (r + 1), :],
                                          in_=ctx4[0:BS, :])

            # ---------- phase 1: gx = x @ WiT + ctx4 ----------
            with nc.named_scope("phase1_gemm"):
                WiT_sb = ph1_big.tile([P, KI, G], F16, tag="WiT_sb")
                for k in range(KI):
                    nc.sync.dma_start(out=WiT_sb[:, k, :], in_=WiT_r[:, k, :])

                n_groups = (n_mtiles + 3) // 4
                for grp in range(n_groups):
                    mt_in_grp = min(4, n_mtiles - grp * 4)
                    x_sb = ph1_x.tile([P, KI, 512], F16, tag="x_sb")
                    nc.sync.dma_start(
                        out=x_sb[:, :, 0:128 * mt_in_grp],
                        in_=xT_r[:, :, 512 * grp:512 * grp + 128 * mt_in_grp])
                    for mi in range(mt_in_grp):
                        m = grp * 4 + mi
                        # batch the whole m-tile's output so the gx write is
                        # one fully-contiguous 1MB transfer (rows of [T,BS,G]
                        # are contiguous) instead of 8 strided 1KB-line DMAs.
                        gxo = ph1_out.tile([P, G], F16, tag="gxo")
                        for n in range(NCH):
                            ps1 = ph1_ps.tile([P, 512], F32, tag="ps1")
                            for k in range(KI):
                                nc.tensor.matmul(
                                    ps1, lhsT=x_sb[:, k, 128 * mi:128 * (mi + 1)],
                                    rhs=WiT_sb[:, k, 512 * n:512 * (n + 1)],
                                    start=(k == 0), stop=(k == KI - 1))
                            if ph1_no_drain:
                                continue
                            nc.vector.scalar_tensor_tensor(
                                out=gxo[:, 512 * n:512 * (n + 1)], in0=ps1,
                                scalar=1.0,
                                in1=ctx4[:, 512 * n:512 * (n + 1)],
                                op0=AOP.mult, op1=AOP.add)
                        if ph1_no_drain or ph1_no_write:
                            continue
                        ph1_wr = getattr(nc, ph1_wr_engine)
                        ph1_wr.dma_start(
                            out=gx_rows[128 * m:128 * (m + 1), :], in_=gxo)

        # ---------- phase 2: recurrence ----------
        with (
            tc.tile_pool(name="p2_w", bufs=1) as p2_w,
            tc.tile_pool(name="p2_state", bufs=1) as p2_state,
            tc.tile_pool(name="p2_gx", bufs=4) as p2_gx,
            tc.tile_pool(name="p2_act", bufs=10) as p2_act,
            tc.tile_pool(name="p2_cell", bufs=4) as p2_cell,
            tc.tile_pool(name="p2_ht", bufs=24) as p2_ht,
        ):
            WhT_sb = p2_w.tile([P, KH, G], F16, tag="WhT_sb")
            for k in range(KH):
                nc.sync.dma_start(out=WhT_sb[:, k, :], in_=WhT_r[:, k, :])
            WfaT_sb = p2_w.tile([P, KH, F], F16, tag="WfaT_sb")
            nc.sync.dma_start(out=WfaT_sb, in_=WfaT_r)
            WfcT_sb = p2_w.tile([P, F // P, NCLS], F16, tag="WfcT_sb")
            nc.sync.dma_start(out=WfcT_sb, in_=WfcT_r)

            c_st = p2_state.tile([BS, H], F32, tag="c_st")        # [32, 1024]
            nc.vector.memset(c_st, 0.0)
            hT = []
            for j in range(2):
                ht0 = p2_ht.tile([P, 4, BS], F16, tag="ht", name="ht0")
                nc.vector.memset(ht0, 0.0)
                hT.extend(ht0[:, q, :] for q in range(4))

            # chunk semantics: ig=0,1 fg=2,3 cg=4,5 og=6,7; half j uses
            # chunks {0+j, 2+j, 4+j, 6+j}. Per-half order ig -> cg -> fg -> og.
            # Each chunk: prologue (gx via identity-matmul into PSUM + k0..3)
            # then completion (k4..7) staggered so chunk completions -- and the
            # ACT drains that read PSUM directly -- spread across the step.
            ALLCH = [0, 4, 2, 6, 1, 5, 3, 7]
            with tc.tile_pool(name="p2_ps", bufs=8, space="PSUM") as p2_ps:
                for t in range(rec_steps):
                    with nc.named_scope("step"):
                        gx_sb = p2_gx.tile([BS, G], F16, tag="gx_sb")
                        nc.gpsimd.dma_start(out=gx_sb, in_=gx[t % t_steps, :, :])

                        ps_t = {}
                        pend = list(ALLCH)

                        def chunk_uses_ident(n):
                            # og chunks drain via ACT-direct (identity-matmul
                            # injects gx in PSUM) to shorten the critical tail;
                            # other chunks use DVE stt drains to spare the PE.
                            if no_gx_mm:
                                return False
                            if not gx_via_dve:
                                return True
                            return n in (6, 7)

                        def prologue(n):
                            ps_t[n] = p2_ps.tile([BS, 512], F32, tag="ps2",
                                                 name="ps2")
                            if chunk_uses_ident(n):
                                nc.tensor.matmul(
                                    ps_t[n], lhsT=ident_sb,
                                    rhs=gx_sb[:, 512 * n:512 * (n + 1)],
                                    start=True, stop=False)
                            for k in range(4):
                                nc.tensor.matmul(
                                    ps_t[n], lhsT=hT[k],
                                    rhs=WhT_sb[:, k, 512 * n:512 * (n + 1)],
                                    start=(not chunk_uses_ident(n) and k == 0),
                                    stop=False)

                        def finish_chunk(n):
                            for k in range(4, KH):
                                nc.tensor.matmul(
                                    ps_t[n], lhsT=hT[k],
                                    rhs=WhT_sb[:, k, 512 * n:512 * (n + 1)],
                                    start=False, stop=(k == KH - 1))
                            if pend:
                                prologue(pend.pop(0))
                            func = AFT.Tanh if n in (4, 5) else AFT.Sigmoid
                            ga = p2_act.tile([BS, 512], F16, tag="ga",
                                             name="ga")
                            if chunk_uses_ident(n) or no_gx_mm:
                                nc.scalar.activation(out=ga, in_=ps_t[n],
                                                     func=func)
                            else:
                                gs = p2_act.tile([BS, 512], F16, tag="gs",
                                                 name="gs")
                                nc.vector.scalar_tensor_tensor(
                                    out=gs, in0=ps_t[n], scalar=1.0,
                                    in1=gx_sb[:, 512 * n:512 * (n + 1)],
                                    op0=AOP.mult, op1=AOP.add)
                                nc.scalar.activation(out=gs if False else ga,
                                                     in_=gs, func=func)
                            return ga

                        for _ in range(3):
                            prologue(pend.pop(0))

                        hT_new = []
                        for j in range(2):        # unit halves 0:512, 512:1024
                            ch = c_st[:, 512 * j:512 * (j + 1)]
                            ig_a = finish_chunk(0 + j)
                            tcg = finish_chunk(4 + j)
                            t2 = p2_cell.tile([BS, 512], F16, tag="t2")
                            nc.vector.tensor_tensor(out=t2, in0=ig_a, in1=tcg,
                                                    op=AOP.mult)
                            fg_a = finish_chunk(2 + j)
                            t1 = p2_cell.tile([BS, 512], F32, tag="t1")
                            nc.vector.tensor_tensor(out=t1, in0=fg_a, in1=ch,
                                                    op=AOP.mult)
                            nc.vector.tensor_tensor(out=ch, in0=t1, in1=t2,
                                                    op=AOP.add)
                            tc_t = p2_cell.tile([BS, 512], F16, tag="tc_t")
                            nc.scalar.activation(out=tc_t, in_=ch, func=AFT.Tanh)
                            og_a = finish_chunk(6 + j)
                            h_half = p2_cell.tile([BS, 512], F16, tag="h_half")
                            nc.vector.tensor_tensor(out=h_half, in0=og_a,
                                                    in1=tc_t, op=AOP.mult)
                            # one xbar transpose: [32,512] -> [128,4,32] with
                            # [:, q, :] = h_T[512j+128q : 512j+128(q+1), :]
                            htn = p2_ht.tile([P, 4, BS], F16, tag="ht",
                                             name="htn")
                            nc.sync.dma_start_transpose(out=htn, in_=h_half)
                            hT_new.extend(htn[:, q, :] for q in range(4))
                        hT = hT_new

            # ---------- head ----------
            with nc.named_scope("head"):
                with tc.tile_pool(name="head_ps", bufs=1, space="PSUM") as hps:
                    ps_f = hps.tile([BS, F], F32, tag="ps_f")
                    for k in range(KH):
                        nc.tensor.matmul(ps_f, lhsT=hT[k],
                                         rhs=WfaT_sb[:, k, :],
                                         start=(k == 0), stop=(k == KH - 1))
                    x1 = p2_cell.tile([BS, F], F32, tag="x1", bufs=1)
                    nc.vector.scalar_tensor_tensor(
                        out=x1, in0=ps_f, scalar=1.0, in1=bfa_rep,
                        op0=AOP.mult, op1=AOP.add)
                    x1r = p2_cell.tile([BS, F], F16, tag="x1r", bufs=1)
                    nc.scalar.activation(out=x1r, in_=x1, func=AFT.Relu)
                    x1T = p2_ht.tile([P, F // P, BS], F16, tag="x1T", bufs=1)
                    nc.sync.dma_start_transpose(out=x1T, in_=x1r)
                    ps_o = hps.tile([BS, NCLS], F32, tag="ps_o")
                    for q in range(F // P):
                        nc.tensor.matmul(ps_o, lhsT=x1T[:, q, :],
                                         rhs=WfcT_sb[:, q, :],
                                         start=(q == 0), stop=(q == F // P - 1))
                    out_sb = p2_cell.tile([BS, NCLS], F32, tag="out_sb", bufs=1)
                    nc.vector.scalar_tensor_tensor(
                        out=out_sb, in0=ps_o, scalar=1.0, in1=bfc_rep,
                        op0=AOP.mult, op1=AOP.add)
                    nc.sync.dma_start(out=out[:], in_=out_sb)

    nc.compile()
    return nc


def make_in_maps(inputs: dict, t_steps: int = T):
    """Shard + lay out the full inputs for the 8 cores (host-side numpy)."""
    x = np.asarray(inputs["i_features"], np.float32)[:, :t_steps, :]
    v = np.asarray(inputs["v_features"], np.float32)
    Wi, bi = np.asarray(inputs["Wi"], np.float32), np.asarray(inputs["bi"], np.float32)
    Wh, bh = np.asarray(inputs["Wh"], np.float32), np.asarray(inputs["bh"], np.float32)
    Wc, bc = np.asarray(inputs["Wc"], np.float32), np.asarray(inputs["bc"], np.float32)
    Wfa, bfa = np.asarray(inputs["Wfa"], np.float32), np.asarray(inputs["bfa"], np.float32)
    Wfc, bfc = np.asarray(inputs["Wfc"], np.float32), np.asarray(inputs["bfc"], np.float32)

    shared = {
        "WiT": np.ascontiguousarray(Wi.T).astype(np.float16),
        "WhT": np.ascontiguousarray(Wh.T).astype(np.float16),
        "WcT": np.ascontiguousarray(Wc.T).astype(np.float16),
        "WfaT": np.ascontiguousarray(Wfa.T).astype(np.float16),
        "WfcT": np.ascontiguousarray(Wfc.T).astype(np.float16),
        "bias_total": (bi + bh + bc).astype(np.float32),
        "bfa": bfa.astype(np.float32),
        "bfc": bfc.astype(np.float32),
        "ident32": np.eye(BS, dtype=np.float16),
    }
    in_maps = []
    nb = x.shape[0] // BS
    for s in range(nb):
        xs = x[s * BS:(s + 1) * BS]                      # [BS, t, I]
        in_maps.append({
            "xT": np.ascontiguousarray(xs.transpose(2, 1, 0)).astype(np.float16),
            "vT": np.ascontiguousarray(v[s * BS:(s + 1) * BS].T).astype(np.float16),
            **shared,
        })
    return in_maps


_NC_CACHE = {}


def kernel(**inputs) -> np.ndarray:
    in_maps = make_in_maps(inputs, T)
    if T not in _NC_CACHE:
        _NC_CACHE[T] = build_lstm(T)
    nc = _NC_CACHE[T]
    res = run_bass_kernel_spmd(nc, in_maps, core_ids=list(range(NCORES)))
    return np.concatenate([r["out"] for r in res.results], axis=0).astype(np.float32)



# revision 5
# speedup vs baseline: 11.3112x; 11.3112x over previous
"""Trainium2 Bass kernel for a context-LSTM decoder.

Model (B=256, T=256, I=H=1024, 4H=4096, F=512, NC=7):
    ctx   = v @ Wc.T + (bc + bi + bh)                      # [B, 4H], const over t
    gx    = i_features @ Wi.T + ctx                        # [B, T, 4H]
    per t: gates = gx[t] + h @ Wh.T ; LSTM cell update
    out   = relu(h_T @ Wfa.T + bfa) @ Wfc.T + bfc          # [B, 7]

Strategy: pure data-parallel over batch, 32 rows per core, no collectives.
All matmul operands fp16 (fp32 PSUM accumulation), cell state c fp32.

Phase 1 (big GEMM) tiles M=(4t x 32b)=128 rows at full PE width.

Phase 2 (recurrence) computes gates TRANSPOSED: gates^T[g, b] =
sum_h WhT[h, g]*hT[h, b] + gx^T[g, b].  The PE holds WhT 128x128 tiles as the
STATIONARY operand (fast-weight-load path: 4 elem/cell/cycle) and streams
hT [128, 32] as the moving operand, so per-step PE cost is load-bound
(~8-16K cycles) instead of streaming all of Wh at 1 col/cycle (32K cycles).
gx is injected into PSUM by a transpose-matmul (gx tile stationary, 32x32
identity moving), which doubles as the [32, G] -> [G-part, 32] transpose, so
gx keeps its contiguous [T, BS, G] DRAM layout for both the phase-1 write and
the per-step read.  h and c live as [128, 8*32] (col = 32k+b), so cell-update
ACT/DVE ops use all 128 lanes and h_new IS the next step's moving operand --
no per-step DMA transposes anywhere.
"""

import numpy as np
from contextlib import ExitStack

import concourse.bass as bass
import concourse.bacc as bacc
import concourse.mybir as mybir
import concourse.tile as tile
from concourse.bass_utils import run_bass_kernel_spmd

B, T, I = 256, 256, 1024
H = 1024
G = 4 * H
F = 512
NCLS = 7
NCORES = 8
BS = B // NCORES          # 32 batch rows per core
P = 128                   # partitions
KI = I // P               # 8 k-tiles over the input dim
KH = H // P               # 8 k-tiles over the hidden dim
NCH = G // 512            # 8 chunks of 512 gate columns
HB = KH * BS              # 256: h/c state free dim (col = 32k + b)
F16 = mybir.dt.float16
F32 = mybir.dt.float32
AOP = mybir.AluOpType
AFT = mybir.ActivationFunctionType


def build_lstm(t_steps: int = T, rec_steps: int | None = None, reps: int = 1):
    # rec_steps: number of recurrence iterations (defaults to t_steps). When
    # larger than t_steps, gx rows are reused cyclically (timing studies only).
    # reps: repeat the whole computation in-NEFF (timing studies only).
    if rec_steps is None:
        rec_steps = t_steps
    assert t_steps % 4 == 0
    n_mtiles = t_steps * BS // P          # phase-1 M-tiles (128 bt-rows each)
    nc = bacc.Bacc("TRN2", target_bir_lowering=False, debug=False,
                   num_devices=NCORES)

    xT = nc.declare_dram_parameter("xT", [I, t_steps, BS], F16, isOutput=False)
    vT = nc.declare_dram_parameter("vT", [I, BS], F16, isOutput=False)
    WiT = nc.declare_dram_parameter("WiT", [I, G], F16, isOutput=False)
    WhT = nc.declare_dram_parameter("WhT", [H, G], F16, isOutput=False)
    WcT = nc.declare_dram_parameter("WcT", [I, G], F16, isOutput=False)
    WfaT = nc.declare_dram_parameter("WfaT", [H, F], F16, isOutput=False)
    WfcT = nc.declare_dram_parameter("WfcT", [F, NCLS], F16, isOutput=False)
    bias = nc.declare_dram_parameter("bias_total", [G], F32, isOutput=False)
    bfa = nc.declare_dram_parameter("bfa", [F], F32, isOutput=False)
    bfc = nc.declare_dram_parameter("bfc", [NCLS], F32, isOutput=False)
    ident = nc.declare_dram_parameter("ident32", [BS, BS], F16, isOutput=False)
    out = nc.declare_dram_parameter("out", [BS, NCLS], F32, isOutput=True)

    gx = nc.dram_tensor("gx", [t_steps, BS, G], F16)

    # K-major views of DRAM tensors: i = k*128 + p
    xT_r = xT[:].rearrange("(k p) t b -> p k (t b)", p=P)
    WiT_r = WiT[:].rearrange("(k p) g -> p k g", p=P)
    WhT_r = WhT[:].rearrange("(k p) g -> p k g", p=P)
    WcT_r = WcT[:].rearrange("(k p) g -> p k g", p=P)
    WfaT_r = WfaT[:].rearrange("(k p) f -> p k f", p=P)
    WfcT_r = WfcT[:].rearrange("(q p) n -> p q n", p=P)
    vT_r = vT[:].rearrange("(k p) b -> p k b", p=P)
    gx_rows = gx[:].rearrange("t b g -> (t b) g")

    def bcast(src_ap, rows):
        # read a [cols] DRAM vector into [rows, cols] SBUF (partition bcast)
        return bass.AP(tensor=src_ap.tensor, offset=src_ap.offset,
                       ap=[[0, rows]] + list(src_ap.ap))

    with tile.TileContext(nc) as tc, ExitStack() as ctx:
        consts = ctx.enter_context(tc.tile_pool(name="consts", bufs=1))

        # ---- small resident constants ----
        bfa_rep = consts.tile([BS, F], F32, tag="bfa_rep")
        nc.sync.dma_start(out=bfa_rep, in_=bcast(bfa[:], BS))
        bfc_rep = consts.tile([BS, NCLS], F32, tag="bfc_rep")
        nc.sync.dma_start(out=bfc_rep, in_=bcast(bfc[:], BS))
        ident_sb = consts.tile([BS, BS], F16, tag="ident_sb")
        nc.sync.dma_start(out=ident_sb, in_=ident[:])

      # (reps > 1 repeats the full computation in-NEFF for timing studies)
        for _rep in range(reps):
            with (
                tc.tile_pool(name="ph1_big", bufs=1) as ph1_big,
                tc.tile_pool(name="ph1_ps", bufs=4, space="PSUM") as ph1_ps,
                tc.tile_pool(name="ph1_x", bufs=2) as ph1_x,
                tc.tile_pool(name="ph1_out", bufs=3) as ph1_out,
            ):
                ctx4 = ph1_big.tile([P, G], F32, tag="ctx4")
                # ------- phase 0: ctx4 = v @ WcT + biases, replicated 4x -------
                with nc.named_scope("phase0_ctx"):
                    with (
                        tc.tile_pool(name="ph0_w", bufs=2) as ph0_w,
                        tc.tile_pool(name="ph0_misc", bufs=1) as ph0_misc,
                    ):
                        v_sb = ph0_misc.tile([P, KI, BS], F16, tag="v_sb")
                        nc.sync.dma_start(out=v_sb, in_=vT_r)
                        bias_rep = ph0_misc.tile([BS, G], F32, tag="bias_rep")
                        nc.sync.dma_start(out=bias_rep, in_=bcast(bias[:], BS))
                        for n in range(NCH):
                            wc_t = ph0_w.tile([P, KI, 512], F16, tag="wc")
                            nc.sync.dma_start(
                                out=wc_t, in_=WcT_r[:, :, 512 * n:512 * (n + 1)])
                            ps0 = ph1_ps.tile([BS, 512], F32, tag="ps0", bufs=2)
                            for k in range(KI):
                                nc.tensor.matmul(ps0, lhsT=v_sb[:, k, :],
                                                 rhs=wc_t[:, k, :],
                                                 start=(k == 0),
                                                 stop=(k == KI - 1))
                            nc.vector.scalar_tensor_tensor(
                                out=ctx4[0:BS, 512 * n:512 * (n + 1)], in0=ps0,
                                scalar=1.0,
                                in1=bias_rep[:, 512 * n:512 * (n + 1)],
                                op0=AOP.mult, op1=AOP.add)
                        for r in range(1, 4):
                            nc.sync.dma_start(out=ctx4[BS * r:BS * (r + 1), :],
                                              in_=ctx4[0:BS, :])

                # ------- phase 1: gx = x @ WiT + ctx4 -------
                with nc.named_scope("phase1_gemm"):
                    WiT_sb = ph1_big.tile([P, KI, G], F16, tag="WiT_sb")
                    for k in range(KI):
                        nc.sync.dma_start(out=WiT_sb[:, k, :], in_=WiT_r[:, k, :])

                    n_groups = (n_mtiles + 3) // 4
                    for grp in range(n_groups):
                        mt_in_grp = min(4, n_mtiles - grp * 4)
                        x_sb = ph1_x.tile([P, KI, 512], F16, tag="x_sb")
                        nc.sync.dma_start(
                            out=x_sb[:, :, 0:128 * mt_in_grp],
                            in_=xT_r[:, :, 512 * grp:512 * grp + 128 * mt_in_grp])
                        for mi in range(mt_in_grp):
                            m = grp * 4 + mi
                            # batch the whole m-tile's output so the gx write is
                            # one fully-contiguous 1MB transfer (rows of [T,BS,G]
                            # are contiguous) instead of 8 strided 1KB-line DMAs.
                            gxo = ph1_out.tile([P, G], F16, tag="gxo")
                            for n in range(NCH):
                                ps1 = ph1_ps.tile([P, 512], F32, tag="ps1")
                                for k in range(KI):
                                    nc.tensor.matmul(
                                        ps1,
                                        lhsT=x_sb[:, k, 128 * mi:128 * (mi + 1)],
                                        rhs=WiT_sb[:, k, 512 * n:512 * (n + 1)],
                                        start=(k == 0), stop=(k == KI - 1))
                                nc.vector.scalar_tensor_tensor(
                                    out=gxo[:, 512 * n:512 * (n + 1)], in0=ps1,
                                    scalar=1.0,
                                    in1=ctx4[:, 512 * n:512 * (n + 1)],
                                    op0=AOP.mult, op1=AOP.add)
                            nc.gpsimd.dma_start(
                                out=gx_rows[128 * m:128 * (m + 1), :], in_=gxo)

            # ------- phase 2: recurrence (Wh stationary, transposed gates) ----
            with (
                tc.tile_pool(name="p2_w", bufs=1) as p2_w,
                tc.tile_pool(name="p2_state", bufs=1) as p2_state,
                tc.tile_pool(name="p2_gx", bufs=4) as p2_gx,
                tc.tile_pool(name="p2_act", bufs=10) as p2_act,
                tc.tile_pool(name="p2_cell", bufs=6) as p2_cell,
                tc.tile_pool(name="p2_ht", bufs=3) as p2_ht,
            ):
                WhT_sb = p2_w.tile([P, KH, G], F16, tag="WhT_sb")
                for k in range(KH):
                    nc.sync.dma_start(out=WhT_sb[:, k, :], in_=WhT_r[:, k, :])
                WfaT_sb = p2_w.tile([P, KH, F], F16, tag="WfaT_sb")
                nc.sync.dma_start(out=WfaT_sb, in_=WfaT_r)
                WfcT_sb = p2_w.tile([P, F // P, NCLS], F16, tag="WfcT_sb")
                nc.sync.dma_start(out=WfcT_sb, in_=WfcT_r)

                c_st = p2_state.tile([P, HB], F32, tag="c_st")   # [128, 256]
                nc.vector.memset(c_st, 0.0)
                h = p2_state.tile([P, HB], F16, tag="h0")
                nc.vector.memset(h, 0.0)

                with tc.tile_pool(name="p2_ps", bufs=4, space="PSUM") as p2_ps:
                    for t in range(rec_steps):
                        with nc.named_scope("step"):
                            gx_sb = p2_gx.tile([BS, G], F16, tag="gx_sb")
                            nc.gpsimd.dma_start(out=gx_sb,
                                                in_=gx[t % t_steps, :, :])
                            ps_if = p2_ps.tile([P, 512], F32, tag="ps_if")
                            ps_co = p2_ps.tile([P, 512], F32, tag="ps_co")

                            def block(goff, ps, coff):
                                # one gate block: 8 m-tiles of 128 gate rows;
                                # m-tile j <-> hidden units [128j, 128j+128),
                                # PSUM cols [coff+32j, coff+32j+32)
                                for j in range(KH):
                                    g0 = goff + P * j
                                    dst = ps[:, coff + BS * j:
                                             coff + BS * (j + 1)]
                                    nc.tensor.matmul(
                                        dst, lhsT=gx_sb[:, g0:g0 + P],
                                        rhs=ident_sb, start=True, stop=False)
                                    for k in range(KH):
                                        nc.tensor.matmul(
                                            dst, lhsT=WhT_sb[:, k, g0:g0 + P],
                                            rhs=h[:, BS * k:BS * (k + 1)],
                                            start=False, stop=(k == KH - 1))

                            block(0, ps_if, 0)          # i gate
                            ig_a = p2_act.tile([P, HB], F16, tag="ig_a")
                            nc.scalar.activation(out=ig_a, in_=ps_if[:, 0:HB],
                                                 func=AFT.Sigmoid)
                            block(2 * H, ps_co, 0)      # c~ (cell candidate)
                            tcg = p2_act.tile([P, HB], F16, tag="tcg")
                            nc.scalar.activation(out=tcg, in_=ps_co[:, 0:HB],
                                                 func=AFT.Tanh)
                            t2 = p2_cell.tile([P, HB], F16, tag="t2")
                            nc.vector.tensor_tensor(out=t2, in0=ig_a, in1=tcg,
                                                    op=AOP.mult)
                            block(H, ps_if, HB)         # f gate
                            fg_a = p2_act.tile([P, HB], F16, tag="fg_a")
                            nc.scalar.activation(out=fg_a,
                                                 in_=ps_if[:, HB:2 * HB],
                                                 func=AFT.Sigmoid)
                            t1 = p2_cell.tile([P, HB], F32, tag="t1")
                            nc.vector.tensor_tensor(out=t1, in0=fg_a, in1=c_st,
                                                    op=AOP.mult)
                            nc.vector.tensor_tensor(out=c_st, in0=t1, in1=t2,
                                                    op=AOP.add)
                            tct = p2_cell.tile([P, HB], F16, tag="tct")
                            nc.scalar.activation(out=tct, in_=c_st,
                                                 func=AFT.Tanh)
                            block(3 * H, ps_co, HB)     # o gate
                            og_a = p2_act.tile([P, HB], F16, tag="og_a")
                            nc.scalar.activation(out=og_a,
                                                 in_=ps_co[:, HB:2 * HB],
                                                 func=AFT.Sigmoid)
                            h_new = p2_ht.tile([P, HB], F16, tag="ht")
                            nc.vector.tensor_tensor(out=h_new, in0=og_a,
                                                    in1=tct, op=AOP.mult)
                            h = h_new

                # ------- head -------
                hT = [h[:, BS * k:BS * (k + 1)] for k in range(KH)]
                with nc.named_scope("head"):
                    with tc.tile_pool(name="head_ps", bufs=1,
                                      space="PSUM") as hps:
                        ps_f = hps.tile([BS, F], F32, tag="ps_f")
                        for k in range(KH):
                            nc.tensor.matmul(ps_f, lhsT=hT[k],
                                             rhs=WfaT_sb[:, k, :],
                                             start=(k == 0), stop=(k == KH - 1))
                        x1 = p2_cell.tile([BS, F], F32, tag="x1", bufs=1)
                        nc.vector.scalar_tensor_tensor(
                            out=x1, in0=ps_f, scalar=1.0, in1=bfa_rep,
                            op0=AOP.mult, op1=AOP.add)
                        x1r = p2_cell.tile([BS, F], F16, tag="x1r", bufs=1)
                        nc.scalar.activation(out=x1r, in_=x1, func=AFT.Relu)
                        x1T = p2_act.tile([P, F // P, BS], F16, tag="x1T",
                                          bufs=1)
                        nc.sync.dma_start_transpose(out=x1T, in_=x1r)
                        ps_o = hps.tile([BS, NCLS], F32, tag="ps_o")
                        for q in range(F // P):
                            nc.tensor.matmul(ps_o, lhsT=x1T[:, q, :],
                                             rhs=WfcT_sb[:, q, :],
                                             start=(q == 0),
                                             stop=(q == F // P - 1))
                        out_sb = p2_cell.tile([BS, NCLS], F32, tag="out_sb",
                                              bufs=1)
                        nc.vector.scalar_tensor_tensor(
                            out=out_sb, in0=ps_o, scalar=1.0, in1=bfc_rep,
                            op0=AOP.mult, op1=AOP.add)
                        nc.sync.dma_start(out=out[:], in_=out_sb)

    nc.compile()
    return nc


def make_in_maps(inputs: dict, t_steps: int = T):
    """Shard + lay out the full inputs for the 8 cores (host-side numpy)."""
    x = np.asarray(inputs["i_features"], np.float32)[:, :t_steps, :]
    v = np.asarray(inputs["v_features"], np.float32)
    Wi, bi = np.asarray(inputs["Wi"], np.float32), np.asarray(inputs["bi"], np.float32)
    Wh, bh = np.asarray(inputs["Wh"], np.float32), np.asarray(inputs["bh"], np.float32)
    Wc, bc = np.asarray(inputs["Wc"], np.float32), np.asarray(inputs["bc"], np.float32)
    Wfa, bfa = np.asarray(inputs["Wfa"], np.float32), np.asarray(inputs["bfa"], np.float32)
    Wfc, bfc = np.asarray(inputs["Wfc"], np.float32), np.asarray(inputs["bfc"], np.float32)

    shared = {
        "WiT": np.ascontiguousarray(Wi.T).astype(np.float16),
        "WhT": np.ascontiguousarray(Wh.T).astype(np.float16),
        "WcT": np.ascontiguousarray(Wc.T).astype(np.float16),
        "WfaT": np.ascontiguousarray(Wfa.T).astype(np.float16),
        "WfcT": np.ascontiguousarray(Wfc.T).astype(np.float16),
        "bias_total": (bi + bh + bc).astype(np.float32),
        "bfa": bfa.astype(np.float32),
        "bfc": bfc.astype(np.float32),
        "ident32": np.eye(BS, dtype=np.float16),
    }
    in_maps = []
    nb = x.shape[0] // BS
    for s in range(nb):
        xs = x[s * BS:(s + 1) * BS]                      # [BS, t, I]
        in_maps.append({
            "xT": np.ascontiguousarray(xs.transpose(2, 1, 0)).astype(np.float16),
            "vT": np.ascontiguousarray(v[s * BS:(s + 1) * BS].T).astype(np.float16),
            **shared,
        })
    return in_maps


_NC_CACHE = {}


def finish_output(per_core_outs: list) -> np.ndarray:
    """Host-side gather of per-core 'out' tensors into the full [B, NC] result."""
    return np.concatenate(per_core_outs, axis=0).astype(np.float32)


def kernel(**inputs) -> np.ndarray:
    in_maps = make_in_maps(inputs, T)
    if T not in _NC_CACHE:
        _NC_CACHE[T] = build_lstm(T)
    nc = _NC_CACHE[T]
    res = run_bass_kernel_spmd(nc, in_maps, core_ids=list(range(NCORES)))
    return finish_output([r["out"] for r in res.results])


# revision 6
# speedup vs baseline: 12.1861x; 1.0773x over previous
"""Trainium2 Bass kernel for a context-LSTM decoder.

Model (B=256, T=256, I=H=1024, 4H=4096, F=512, NC=7):
    ctx   = v @ Wc.T + (bc + bi + bh)                      # [B, 4H], const over t
    gx    = i_features @ Wi.T + ctx                        # [B, T, 4H]
    per t: gates = gx[t] + h @ Wh.T ; LSTM cell update
    out   = relu(h_T @ Wfa.T + bfa) @ Wfc.T + bfc          # [B, 7]

Strategy: pure data-parallel over batch, 32 rows per core, no collectives.
Phase-1 GEMM operands fp16 (fp32 PSUM accumulation), cell state c fp32.

Phase 2 streams Wh through the PE as the moving operand.  In fp16 that costs
1 column/cycle (32768 cycles/step); with Wh and h_T in fp8e4 and
perf_mode=DoubleRow the PE consumes TWO interleaved k-rows per cycle
(contraction 256 per matmul), halving the per-step floor to ~16384 cycles.
gx keeps fp16 precision (it carries the large input-projection term), which
bounds the fp8 error to the small h@Wh.T term.  The per-step h -> h_T
transpose stays fp16 (1-byte DMA transpose unsupported); an ACT copy converts
h_T to fp8 off the critical PE path.
"""

import numpy as np
from contextlib import ExitStack

import concourse.bass as bass
import concourse.bacc as bacc
import concourse.mybir as mybir
import concourse.tile as tile
from concourse.bass_utils import run_bass_kernel_spmd

B, T, I = 256, 256, 1024
H = 1024
G = 4 * H
F = 512
NCLS = 7
NCORES = 8
BS = B // NCORES          # 32 batch rows per core
P = 128                   # partitions
KI = I // P               # 8 k-tiles over the input dim
KH = H // P               # 8 k-tiles over the hidden dim
NCH = G // 512            # 8 chunks of 512 gate columns
F16 = mybir.dt.float16
F32 = mybir.dt.float32
F8 = mybir.dt.float8e4
AOP = mybir.AluOpType
AFT = mybir.ActivationFunctionType
DR = mybir.MatmulPerfMode.DoubleRow

WH_FP8 = True             # fp8e4 + DoubleRow for the recurrence h @ Wh.T


def build_lstm(t_steps: int = T, rec_steps: int | None = None, reps: int = 1,
               wh_fp8: bool = WH_FP8):
    # rec_steps: number of recurrence iterations (defaults to t_steps). When
    # larger than t_steps, gx rows are reused cyclically (timing studies only).
    # reps: repeat the whole computation in-NEFF (timing studies only).
    if rec_steps is None:
        rec_steps = t_steps
    assert t_steps % 4 == 0
    n_mtiles = t_steps * BS // P          # phase-1 M-tiles (128 bt-rows each)
    nc = bacc.Bacc("TRN2", target_bir_lowering=False, debug=False,
                   num_devices=NCORES)
    WHDT = F8 if wh_fp8 else F16

    xT = nc.declare_dram_parameter("xT", [I, t_steps, BS], F16, isOutput=False)
    vT = nc.declare_dram_parameter("vT", [I, BS], F16, isOutput=False)
    WiT = nc.declare_dram_parameter("WiT", [I, G], F16, isOutput=False)
    WhT = nc.declare_dram_parameter("WhT", [H, G], WHDT, isOutput=False)
    WcT = nc.declare_dram_parameter("WcT", [I, G], F16, isOutput=False)
    WfaT = nc.declare_dram_parameter("WfaT", [H, F], F16, isOutput=False)
    WfcT = nc.declare_dram_parameter("WfcT", [F, NCLS], F16, isOutput=False)
    bias = nc.declare_dram_parameter("bias_total", [G], F32, isOutput=False)
    bfa = nc.declare_dram_parameter("bfa", [F], F32, isOutput=False)
    bfc = nc.declare_dram_parameter("bfc", [NCLS], F32, isOutput=False)
    ident = nc.declare_dram_parameter("ident32", [BS, BS], F16, isOutput=False)
    out = nc.declare_dram_parameter("out", [BS, NCLS], F32, isOutput=True)

    gx = nc.dram_tensor("gx", [t_steps, BS, G], F16)

    # K-major views of DRAM tensors: i = k*128 + p
    xT_r = xT[:].rearrange("(k p) t b -> p k (t b)", p=P)
    WiT_r = WiT[:].rearrange("(k p) g -> p k g", p=P)
    WhT_r = WhT[:].rearrange("(k p) g -> p k g", p=P)
    WcT_r = WcT[:].rearrange("(k p) g -> p k g", p=P)
    WfaT_r = WfaT[:].rearrange("(k p) f -> p k f", p=P)
    WfcT_r = WfcT[:].rearrange("(q p) n -> p q n", p=P)
    vT_r = vT[:].rearrange("(k p) b -> p k b", p=P)
    gx_rows = gx[:].rearrange("t b g -> (t b) g")

    def bcast(src_ap, rows):
        # read a [cols] DRAM vector into [rows, cols] SBUF (partition bcast)
        return bass.AP(tensor=src_ap.tensor, offset=src_ap.offset,
                       ap=[[0, rows]] + list(src_ap.ap))

    with tile.TileContext(nc) as tc, ExitStack() as ctx:
        consts = ctx.enter_context(tc.tile_pool(name="consts", bufs=1))

        # ---- small resident constants ----
        bfa_rep = consts.tile([BS, F], F32, tag="bfa_rep")
        nc.sync.dma_start(out=bfa_rep, in_=bcast(bfa[:], BS))
        bfc_rep = consts.tile([BS, NCLS], F32, tag="bfc_rep")
        nc.sync.dma_start(out=bfc_rep, in_=bcast(bfc[:], BS))
        ident_sb = consts.tile([BS, BS], F16, tag="ident_sb")
        nc.sync.dma_start(out=ident_sb, in_=ident[:])

        for _rep in range(reps):
            with (
                tc.tile_pool(name="ph1_big", bufs=1) as ph1_big,
                tc.tile_pool(name="ph1_ps", bufs=4, space="PSUM") as ph1_ps,
                tc.tile_pool(name="ph1_x", bufs=2) as ph1_x,
                tc.tile_pool(name="ph1_out", bufs=3) as ph1_out,
            ):
                ctx4 = ph1_big.tile([P, G], F32, tag="ctx4")
                # ------- phase 0: ctx4 = v @ WcT + biases, replicated 4x -------
                with nc.named_scope("phase0_ctx"):
                    with (
                        tc.tile_pool(name="ph0_w", bufs=2) as ph0_w,
                        tc.tile_pool(name="ph0_misc", bufs=1) as ph0_misc,
                    ):
                        v_sb = ph0_misc.tile([P, KI, BS], F16, tag="v_sb")
                        nc.sync.dma_start(out=v_sb, in_=vT_r)
                        bias_rep = ph0_misc.tile([BS, G], F32, tag="bias_rep")
                        nc.sync.dma_start(out=bias_rep, in_=bcast(bias[:], BS))
                        for n in range(NCH):
                            wc_t = ph0_w.tile([P, KI, 512], F16, tag="wc")
                            nc.sync.dma_start(
                                out=wc_t, in_=WcT_r[:, :, 512 * n:512 * (n + 1)])
                            ps0 = ph1_ps.tile([BS, 512], F32, tag="ps0", bufs=2)
                            for k in range(KI):
                                nc.tensor.matmul(ps0, lhsT=v_sb[:, k, :],
                                                 rhs=wc_t[:, k, :],
                                                 start=(k == 0),
                                                 stop=(k == KI - 1))
                            nc.vector.scalar_tensor_tensor(
                                out=ctx4[0:BS, 512 * n:512 * (n + 1)], in0=ps0,
                                scalar=1.0,
                                in1=bias_rep[:, 512 * n:512 * (n + 1)],
                                op0=AOP.mult, op1=AOP.add)
                        for r in range(1, 4):
                            nc.sync.dma_start(out=ctx4[BS * r:BS * (r + 1), :],
                                              in_=ctx4[0:BS, :])

                # ------- phase 1: gx = x @ WiT + ctx4 -------
                with nc.named_scope("phase1_gemm"):
                    WiT_sb = ph1_big.tile([P, KI, G], F16, tag="WiT_sb")
                    for k in range(KI):
                        nc.sync.dma_start(out=WiT_sb[:, k, :], in_=WiT_r[:, k, :])

                    n_groups = (n_mtiles + 3) // 4
                    for grp in range(n_groups):
                        mt_in_grp = min(4, n_mtiles - grp * 4)
                        x_sb = ph1_x.tile([P, KI, 512], F16, tag="x_sb")
                        nc.sync.dma_start(
                            out=x_sb[:, :, 0:128 * mt_in_grp],
                            in_=xT_r[:, :, 512 * grp:512 * grp + 128 * mt_in_grp])
                        for mi in range(mt_in_grp):
                            m = grp * 4 + mi
                            # batch the whole m-tile's output so the gx write is
                            # one fully-contiguous 1MB transfer (rows of [T,BS,G]
                            # are contiguous) instead of 8 strided 1KB-line DMAs.
                            gxo = ph1_out.tile([P, G], F16, tag="gxo")
                            for n in range(NCH):
                                ps1 = ph1_ps.tile([P, 512], F32, tag="ps1")
                                for k in range(KI):
                                    nc.tensor.matmul(
                                        ps1,
                                        lhsT=x_sb[:, k, 128 * mi:128 * (mi + 1)],
                                        rhs=WiT_sb[:, k, 512 * n:512 * (n + 1)],
                                        start=(k == 0), stop=(k == KI - 1))
                                nc.vector.scalar_tensor_tensor(
                                    out=gxo[:, 512 * n:512 * (n + 1)], in0=ps1,
                                    scalar=1.0,
                                    in1=ctx4[:, 512 * n:512 * (n + 1)],
                                    op0=AOP.mult, op1=AOP.add)
                            nc.gpsimd.dma_start(
                                out=gx_rows[128 * m:128 * (m + 1), :], in_=gxo)

            # ------- phase 2: recurrence -------
            with (
                tc.tile_pool(name="p2_w", bufs=1) as p2_w,
                tc.tile_pool(name="p2_state", bufs=1) as p2_state,
                tc.tile_pool(name="p2_gx", bufs=4) as p2_gx,
                tc.tile_pool(name="p2_act", bufs=10) as p2_act,
                tc.tile_pool(name="p2_cell", bufs=4) as p2_cell,
                tc.tile_pool(name="p2_ht", bufs=12) as p2_ht,
                tc.tile_pool(name="p2_ht8", bufs=12) as p2_ht8,
            ):
                WhT_sb = p2_w.tile([P, KH, G], WHDT, tag="WhT_sb")
                for k in range(KH):
                    nc.sync.dma_start(out=WhT_sb[:, k, :], in_=WhT_r[:, k, :])
                WfaT_sb = p2_w.tile([P, KH, F], F16, tag="WfaT_sb")
                nc.sync.dma_start(out=WfaT_sb, in_=WfaT_r)
                WfcT_sb = p2_w.tile([P, F // P, NCLS], F16, tag="WfcT_sb")
                nc.sync.dma_start(out=WfcT_sb, in_=WfcT_r)

                c_st = p2_state.tile([BS, H], F32, tag="c_st")        # [32, 1024]
                nc.vector.memset(c_st, 0.0)
                # hT16: fp16 transposed h (also feeds the head); hTp: what the
                # Wh matmuls consume -- fp8 pair-tiles (DoubleRow) or the fp16
                # slices themselves.
                hT16 = []
                hTp = []
                for j in range(2):
                    ht0 = p2_ht.tile([P, 4, BS], F16, tag="ht", name="ht0")
                    nc.vector.memset(ht0, 0.0)
                    hT16.extend(ht0[:, q, :] for q in range(4))
                    if wh_fp8:
                        ht8 = p2_ht8.tile([P, 4, BS], F8, tag="ht8", name="ht80")
                        nc.vector.memset(ht8, 0.0)
                        hTp.extend([ht8[:, 0:2, :], ht8[:, 2:4, :]])
                    else:
                        hTp.extend([ht0[:, q, :] for q in range(4)])

                # chunk semantics: ig=0,1 fg=2,3 cg=4,5 og=6,7; half j uses
                # chunks {0+j, 2+j, 4+j, 6+j}. Per-half order ig -> cg -> fg -> og.
                # Each chunk: prologue (gx via identity-matmul into PSUM + first
                # half of the contraction) then completion (second half),
                # staggered so chunk completions -- and the ACT drains that read
                # PSUM directly -- spread across the step.
                ALLCH = [0, 4, 2, 6, 1, 5, 3, 7]
                with tc.tile_pool(name="p2_ps", bufs=8, space="PSUM") as p2_ps:
                    for t in range(rec_steps):
                        with nc.named_scope("step"):
                            gx_sb = p2_gx.tile([BS, G], F16, tag="gx_sb")
                            nc.gpsimd.dma_start(out=gx_sb,
                                                in_=gx[t % t_steps, :, :])

                            ps_t = {}
                            pend = list(ALLCH)

                            def chunk_uses_ident(n):
                                # og chunks drain via ACT-direct (identity-matmul
                                # injects gx in PSUM) to shorten the critical
                                # tail; others use DVE stt drains.
                                return n in (6, 7)

                            def wh_mms(n, lo, hi, first):
                                # contraction segment [lo, hi) in k-pair units
                                # (fp8 DoubleRow) or k units (fp16)
                                col = slice(512 * n, 512 * (n + 1))
                                if wh_fp8:
                                    for i in range(lo // 2, hi // 2):
                                        nc.tensor.matmul(
                                            ps_t[n], lhsT=hTp[i],
                                            rhs=WhT_sb[:, 2 * i:2 * i + 2, col],
                                            perf_mode=DR,
                                            start=(first and i == lo // 2),
                                            stop=(hi == KH and i == KH // 2 - 1))
                                else:
                                    for k in range(lo, hi):
                                        nc.tensor.matmul(
                                            ps_t[n], lhsT=hTp[k],
                                            rhs=WhT_sb[:, k, col],
                                            start=(first and k == lo),
                                            stop=(k == KH - 1))

                            def prologue(n):
                                ps_t[n] = p2_ps.tile([BS, 512], F32, tag="ps2",
                                                     name="ps2")
                                if chunk_uses_ident(n):
                                    nc.tensor.matmul(
                                        ps_t[n], lhsT=ident_sb,
                                        rhs=gx_sb[:, 512 * n:512 * (n + 1)],
                                        start=True, stop=False)
                                wh_mms(n, 0, 4, first=not chunk_uses_ident(n))

                            def finish_chunk(n):
                                wh_mms(n, 4, KH, first=False)
                                if pend:
                                    prologue(pend.pop(0))
                                func = AFT.Tanh if n in (4, 5) else AFT.Sigmoid
                                ga = p2_act.tile([BS, 512], F16, tag="ga",
                                                 name="ga")
                                if chunk_uses_ident(n):
                                    nc.scalar.activation(out=ga, in_=ps_t[n],
                                                         func=func)
                                else:
                                    gs = p2_act.tile([BS, 512], F16, tag="gs",
                                                     name="gs")
                                    nc.vector.scalar_tensor_tensor(
                                        out=gs, in0=ps_t[n], scalar=1.0,
                                        in1=gx_sb[:, 512 * n:512 * (n + 1)],
                                        op0=AOP.mult, op1=AOP.add)
                                    nc.scalar.activation(out=ga, in_=gs,
                                                         func=func)
                                return ga

                            for _ in range(3):
                                prologue(pend.pop(0))

                            hT16_new, hTp_new = [], []
                            for j in range(2):    # unit halves 0:512, 512:1024
                                ch = c_st[:, 512 * j:512 * (j + 1)]
                                ig_a = finish_chunk(0 + j)
                                tcg = finish_chunk(4 + j)
                                t2 = p2_cell.tile([BS, 512], F16, tag="t2")
                                nc.vector.tensor_tensor(out=t2, in0=ig_a,
                                                        in1=tcg, op=AOP.mult)
                                fg_a = finish_chunk(2 + j)
                                t1 = p2_cell.tile([BS, 512], F32, tag="t1")
                                nc.vector.tensor_tensor(out=t1, in0=fg_a,
                                                        in1=ch, op=AOP.mult)
                                nc.vector.tensor_tensor(out=ch, in0=t1, in1=t2,
                                                        op=AOP.add)
                                tc_t = p2_cell.tile([BS, 512], F16, tag="tc_t")
                                nc.scalar.activation(out=tc_t, in_=ch,
                                                     func=AFT.Tanh)
                                og_a = finish_chunk(6 + j)
                                h_half = p2_cell.tile([BS, 512], F16,
                                                      tag="h_half")
                                nc.vector.tensor_tensor(out=h_half, in0=og_a,
                                                        in1=tc_t, op=AOP.mult)
                                # one xbar transpose: [32,512] -> [128,4,32] with
                                # [:, q, :] = h_T[512j+128q : 512j+128(q+1), :]
                                htn = p2_ht.tile([P, 4, BS], F16, tag="ht",
                                                 name="htn")
                                nc.sync.dma_start_transpose(out=htn, in_=h_half)
                                hT16_new.extend(htn[:, q, :] for q in range(4))
                                if wh_fp8:
                                    ht8 = p2_ht8.tile([P, 4, BS], F8, tag="ht8",
                                                      name="ht8n")
                                    nc.scalar.activation(out=ht8, in_=htn,
                                                         func=AFT.Copy)
                                    hTp_new.extend([ht8[:, 0:2, :],
                                                    ht8[:, 2:4, :]])
                                else:
                                    hTp_new.extend(htn[:, q, :]
                                                   for q in range(4))
                            hT16, hTp = hT16_new, hTp_new

                # ------- head -------
                with nc.named_scope("head"):
                    with tc.tile_pool(name="head_ps", bufs=1,
                                      space="PSUM") as hps:
                        ps_f = hps.tile([BS, F], F32, tag="ps_f")
                        for k in range(KH):
                            nc.tensor.matmul(ps_f, lhsT=hT16[k],
                                             rhs=WfaT_sb[:, k, :],
                                             start=(k == 0), stop=(k == KH - 1))
                        x1 = p2_cell.tile([BS, F], F32, tag="x1", bufs=1)
                        nc.vector.scalar_tensor_tensor(
                            out=x1, in0=ps_f, scalar=1.0, in1=bfa_rep,
                            op0=AOP.mult, op1=AOP.add)
                        x1r = p2_cell.tile([BS, F], F16, tag="x1r", bufs=1)
                        nc.scalar.activation(out=x1r, in_=x1, func=AFT.Relu)
                        x1T = p2_ht.tile([P, F // P, BS], F16, tag="x1T",
                                         bufs=1)
                        nc.sync.dma_start_transpose(out=x1T, in_=x1r)
                        ps_o = hps.tile([BS, NCLS], F32, tag="ps_o")
                        for q in range(F // P):
                            nc.tensor.matmul(ps_o, lhsT=x1T[:, q, :],
                                             rhs=WfcT_sb[:, q, :],
                                             start=(q == 0),
                                             stop=(q == F // P - 1))
                        out_sb = p2_cell.tile([BS, NCLS], F32, tag="out_sb",
                                              bufs=1)
                        nc.vector.scalar_tensor_tensor(
                            out=out_sb, in0=ps_o, scalar=1.0, in1=bfc_rep,
                            op0=AOP.mult, op1=AOP.add)
                        nc.sync.dma_start(out=out[:], in_=out_sb)

    nc.compile()
    return nc


def make_in_maps(inputs: dict, t_steps: int = T, wh_fp8: bool = WH_FP8):
    """Shard + lay out the full inputs for the 8 cores (host-side numpy)."""
    x = np.asarray(inputs["i_features"], np.float32)[:, :t_steps, :]
    v = np.asarray(inputs["v_features"], np.float32)
    Wi, bi = np.asarray(inputs["Wi"], np.float32), np.asarray(inputs["bi"], np.float32)
    Wh, bh = np.asarray(inputs["Wh"], np.float32), np.asarray(inputs["bh"], np.float32)
    Wc, bc = np.asarray(inputs["Wc"], np.float32), np.asarray(inputs["bc"], np.float32)
    Wfa, bfa = np.asarray(inputs["Wfa"], np.float32), np.asarray(inputs["bfa"], np.float32)
    Wfc, bfc = np.asarray(inputs["Wfc"], np.float32), np.asarray(inputs["bfc"], np.float32)

    whdt = mybir.dt.np(F8) if wh_fp8 else np.float16
    shared = {
        "WiT": np.ascontiguousarray(Wi.T).astype(np.float16),
        "WhT": np.ascontiguousarray(Wh.T).astype(whdt),
        "WcT": np.ascontiguousarray(Wc.T).astype(np.float16),
        "WfaT": np.ascontiguousarray(Wfa.T).astype(np.float16),
        "WfcT": np.ascontiguousarray(Wfc.T).astype(np.float16),
        "bias_total": (bi + bh + bc).astype(np.float32),
        "bfa": bfa.astype(np.float32),
        "bfc": bfc.astype(np.float32),
        "ident32": np.eye(BS, dtype=np.float16),
    }
    in_maps = []
    nb = x.shape[0] // BS
    for s in range(nb):
        xs = x[s * BS:(s + 1) * BS]                      # [BS, t, I]
        in_maps.append({
            "xT": np.ascontiguousarray(xs.transpose(2, 1, 0)).astype(np.float16),
            "vT": np.ascontiguousarray(v[s * BS:(s + 1) * BS].T).astype(np.float16),
            **shared,
        })
    return in_maps


_NC_CACHE = {}


def finish_output(per_core_outs: list) -> np.ndarray:
    """Host-side gather of per-core 'out' tensors into the full [B, NC] result."""
    return np.concatenate(per_core_outs, axis=0).astype(np.float32)


def kernel(**inputs) -> np.ndarray:
    in_maps = make_in_maps(inputs, T)
    if T not in _NC_CACHE:
        _NC_CACHE[T] = build_lstm(T)
    nc = _NC_CACHE[T]
    res = run_bass_kernel_spmd(nc, in_maps, core_ids=list(range(NCORES)))
    return finish_output([r["out"] for r in res.results])


# revision 7
# speedup vs baseline: 13.8872x; 1.1396x over previous
"""Trainium2 Bass kernel for a context-LSTM decoder.

Model (B=256, T=256, I=H=1024, 4H=4096, F=512, NC=7):
    ctx   = v @ Wc.T + (bc + bi + bh)                      # [B, 4H], const over t
    gx    = i_features @ Wi.T + ctx                        # [B, T, 4H]
    per t: gates = gx[t] + h @ Wh.T ; LSTM cell update
    out   = relu(h_T @ Wfa.T + bfa) @ Wfc.T + bfc          # [B, 7]

Strategy: pure data-parallel over batch, 32 rows per core, no collectives.
Phase-1 GEMM operands fp16 (fp32 PSUM accumulation), cell state c fp32.

Phase 2 streams Wh through the PE as the moving operand.  In fp16 that costs
1 column/cycle (32768 cycles/step); with Wh and h_T in fp8e4 and
perf_mode=DoubleRow the PE consumes TWO interleaved k-rows per cycle
(contraction 256 per matmul), halving the per-step floor to ~16384 cycles.
gx keeps fp16 precision (it carries the large input-projection term), which
bounds the fp8 error to the small h@Wh.T term.  The per-step h -> h_T
transpose stays fp16 (1-byte DMA transpose unsupported); an ACT copy converts
h_T to fp8 off the critical PE path.
"""

import numpy as np
from contextlib import ExitStack

import concourse.bass as bass
import concourse.bacc as bacc
import concourse.mybir as mybir
import concourse.tile as tile
from concourse.bass_utils import run_bass_kernel_spmd

B, T, I = 256, 256, 1024
H = 1024
G = 4 * H
F = 512
NCLS = 7
NCORES = 8
BS = B // NCORES          # 32 batch rows per core
P = 128                   # partitions
KI = I // P               # 8 k-tiles over the input dim
KH = H // P               # 8 k-tiles over the hidden dim
NCH = G // 512            # 8 chunks of 512 gate columns
F16 = mybir.dt.float16
F32 = mybir.dt.float32
F8 = mybir.dt.float8e4
AOP = mybir.AluOpType
AFT = mybir.ActivationFunctionType
DR = mybir.MatmulPerfMode.DoubleRow

# fp8e4 + DoubleRow for the recurrence h @ Wh.T.  Measured on hardware it is
# a net LOSS (15.9us/step vs 13.2us/step fp16; DoubleRow's weight-load penalty
# and the extra fp16->fp8 converts outweigh the 2-rows/cycle streaming), and
# it costs 12x in accuracy (rel err 8e-3 vs 6.4e-4) -- so it stays off.
WH_FP8 = False


def build_lstm(t_steps: int = T, rec_steps: int | None = None, reps: int = 1,
               wh_fp8: bool = WH_FP8):
    # rec_steps: number of recurrence iterations (defaults to t_steps). When
    # larger than t_steps, gx rows are reused cyclically (timing studies only).
    # reps: repeat the whole computation in-NEFF (timing studies only).
    if rec_steps is None:
        rec_steps = t_steps
    assert t_steps % 4 == 0
    n_mtiles = t_steps * BS // P          # phase-1 M-tiles (128 bt-rows each)
    nc = bacc.Bacc("TRN2", target_bir_lowering=False, debug=False,
                   num_devices=NCORES)
    WHDT = F8 if wh_fp8 else F16

    xT = nc.declare_dram_parameter("xT", [I, t_steps, BS], F16, isOutput=False)
    vT = nc.declare_dram_parameter("vT", [I, BS], F16, isOutput=False)
    WiT = nc.declare_dram_parameter("WiT", [I, G], F16, isOutput=False)
    WhT = nc.declare_dram_parameter("WhT", [H, G], WHDT, isOutput=False)
    WcT = nc.declare_dram_parameter("WcT", [I, G], F16, isOutput=False)
    WfaT = nc.declare_dram_parameter("WfaT", [H, F], F16, isOutput=False)
    WfcT = nc.declare_dram_parameter("WfcT", [F, NCLS], F16, isOutput=False)
    bias = nc.declare_dram_parameter("bias_total", [G], F32, isOutput=False)
    bfa = nc.declare_dram_parameter("bfa", [F], F32, isOutput=False)
    bfc = nc.declare_dram_parameter("bfc", [NCLS], F32, isOutput=False)
    ident = nc.declare_dram_parameter("ident32", [BS, BS], F16, isOutput=False)
    out = nc.declare_dram_parameter("out", [BS, NCLS], F32, isOutput=True)

    gx = nc.dram_tensor("gx", [t_steps, BS, G], F16)

    # K-major views of DRAM tensors: i = k*128 + p
    xT_r = xT[:].rearrange("(k p) t b -> p k (t b)", p=P)
    WiT_r = WiT[:].rearrange("(k p) g -> p k g", p=P)
    WhT_r = WhT[:].rearrange("(k p) g -> p k g", p=P)
    WcT_r = WcT[:].rearrange("(k p) g -> p k g", p=P)
    WfaT_r = WfaT[:].rearrange("(k p) f -> p k f", p=P)
    WfcT_r = WfcT[:].rearrange("(q p) n -> p q n", p=P)
    vT_r = vT[:].rearrange("(k p) b -> p k b", p=P)
    gx_rows = gx[:].rearrange("t b g -> (t b) g")

    def bcast(src_ap, rows):
        # read a [cols] DRAM vector into [rows, cols] SBUF (partition bcast)
        return bass.AP(tensor=src_ap.tensor, offset=src_ap.offset,
                       ap=[[0, rows]] + list(src_ap.ap))

    with tile.TileContext(nc) as tc, ExitStack() as ctx:
        consts = ctx.enter_context(tc.tile_pool(name="consts", bufs=1))

        # ---- small resident constants ----
        bfa_rep = consts.tile([BS, F], F32, tag="bfa_rep")
        nc.sync.dma_start(out=bfa_rep, in_=bcast(bfa[:], BS))
        bfc_rep = consts.tile([BS, NCLS], F32, tag="bfc_rep")
        nc.sync.dma_start(out=bfc_rep, in_=bcast(bfc[:], BS))
        ident_sb = consts.tile([BS, BS], F16, tag="ident_sb")
        nc.sync.dma_start(out=ident_sb, in_=ident[:])

        for _rep in range(reps):
            with (
                tc.tile_pool(name="ph1_big", bufs=1) as ph1_big,
                tc.tile_pool(name="ph1_ps", bufs=4, space="PSUM") as ph1_ps,
                tc.tile_pool(name="ph1_x", bufs=2) as ph1_x,
                tc.tile_pool(name="ph1_out", bufs=3) as ph1_out,
            ):
                ctx4 = ph1_big.tile([P, G], F32, tag="ctx4")
                # ------- phase 0: ctx4 = v @ WcT + biases, replicated 4x -------
                with nc.named_scope("phase0_ctx"):
                    with (
                        tc.tile_pool(name="ph0_w", bufs=2) as ph0_w,
                        tc.tile_pool(name="ph0_misc", bufs=1) as ph0_misc,
                    ):
                        v_sb = ph0_misc.tile([P, KI, BS], F16, tag="v_sb")
                        nc.sync.dma_start(out=v_sb, in_=vT_r)
                        bias_rep = ph0_misc.tile([BS, G], F32, tag="bias_rep")
                        nc.sync.dma_start(out=bias_rep, in_=bcast(bias[:], BS))
                        for n in range(NCH):
                            wc_t = ph0_w.tile([P, KI, 512], F16, tag="wc")
                            nc.sync.dma_start(
                                out=wc_t, in_=WcT_r[:, :, 512 * n:512 * (n + 1)])
                            ps0 = ph1_ps.tile([BS, 512], F32, tag="ps0", bufs=2)
                            for k in range(KI):
                                nc.tensor.matmul(ps0, lhsT=v_sb[:, k, :],
                                                 rhs=wc_t[:, k, :],
                                                 start=(k == 0),
                                                 stop=(k == KI - 1))
                            nc.vector.scalar_tensor_tensor(
                                out=ctx4[0:BS, 512 * n:512 * (n + 1)], in0=ps0,
                                scalar=1.0,
                                in1=bias_rep[:, 512 * n:512 * (n + 1)],
                                op0=AOP.mult, op1=AOP.add)
                        for r in range(1, 4):
                            nc.sync.dma_start(out=ctx4[BS * r:BS * (r + 1), :],
                                              in_=ctx4[0:BS, :])

                # ------- phase 1: gx = x @ WiT + ctx4 -------
                with nc.named_scope("phase1_gemm"):
                    WiT_sb = ph1_big.tile([P, KI, G], F16, tag="WiT_sb")
                    for k in range(KI):
                        nc.sync.dma_start(out=WiT_sb[:, k, :], in_=WiT_r[:, k, :])

                    n_groups = (n_mtiles + 3) // 4
                    for grp in range(n_groups):
                        mt_in_grp = min(4, n_mtiles - grp * 4)
                        x_sb = ph1_x.tile([P, KI, 512], F16, tag="x_sb")
                        nc.sync.dma_start(
                            out=x_sb[:, :, 0:128 * mt_in_grp],
                            in_=xT_r[:, :, 512 * grp:512 * grp + 128 * mt_in_grp])
                        for mi in range(mt_in_grp):
                            m = grp * 4 + mi
                            # batch the whole m-tile's output so the gx write is
                            # one fully-contiguous 1MB transfer (rows of [T,BS,G]
                            # are contiguous) instead of 8 strided 1KB-line DMAs.
                            gxo = ph1_out.tile([P, G], F16, tag="gxo")
                            for n in range(NCH):
                                ps1 = ph1_ps.tile([P, 512], F32, tag="ps1")
                                for k in range(KI):
                                    nc.tensor.matmul(
                                        ps1,
                                        lhsT=x_sb[:, k, 128 * mi:128 * (mi + 1)],
                                        rhs=WiT_sb[:, k, 512 * n:512 * (n + 1)],
                                        start=(k == 0), stop=(k == KI - 1))
                                nc.vector.scalar_tensor_tensor(
                                    out=gxo[:, 512 * n:512 * (n + 1)], in0=ps1,
                                    scalar=1.0,
                                    in1=ctx4[:, 512 * n:512 * (n + 1)],
                                    op0=AOP.mult, op1=AOP.add)
                            nc.gpsimd.dma_start(
                                out=gx_rows[128 * m:128 * (m + 1), :], in_=gxo)

            # ------- phase 2: recurrence -------
            with (
                tc.tile_pool(name="p2_w", bufs=1) as p2_w,
                tc.tile_pool(name="p2_state", bufs=1) as p2_state,
                tc.tile_pool(name="p2_gx", bufs=4) as p2_gx,
                tc.tile_pool(name="p2_act", bufs=10) as p2_act,
                tc.tile_pool(name="p2_cell", bufs=4) as p2_cell,
                tc.tile_pool(name="p2_ht", bufs=12) as p2_ht,
                tc.tile_pool(name="p2_ht8", bufs=12) as p2_ht8,
            ):
                WhT_sb = p2_w.tile([P, KH, G], WHDT, tag="WhT_sb")
                for k in range(KH):
                    nc.sync.dma_start(out=WhT_sb[:, k, :], in_=WhT_r[:, k, :])
                WfaT_sb = p2_w.tile([P, KH, F], F16, tag="WfaT_sb")
                nc.sync.dma_start(out=WfaT_sb, in_=WfaT_r)
                WfcT_sb = p2_w.tile([P, F // P, NCLS], F16, tag="WfcT_sb")
                nc.sync.dma_start(out=WfcT_sb, in_=WfcT_r)

                c_st = p2_state.tile([BS, H], F32, tag="c_st")        # [32, 1024]
                nc.vector.memset(c_st, 0.0)
                # hT16: fp16 transposed h (also feeds the head); hTp: what the
                # Wh matmuls consume -- fp8 pair-tiles (DoubleRow) or the fp16
                # slices themselves.
                hT16 = []
                hTp = []
                for j in range(2):
                    ht0 = p2_ht.tile([P, 4, BS], F16, tag="ht", name="ht0")
                    nc.vector.memset(ht0, 0.0)
                    hT16.extend(ht0[:, q, :] for q in range(4))
                    if wh_fp8:
                        ht8 = p2_ht8.tile([P, 4, BS], F8, tag="ht8", name="ht80")
                        nc.vector.memset(ht8, 0.0)
                        hTp.extend([ht8[:, 0:2, :], ht8[:, 2:4, :]])
                    else:
                        hTp.extend([ht0[:, q, :] for q in range(4)])

                # chunk semantics: ig=0,1 fg=2,3 cg=4,5 og=6,7; half j uses
                # chunks {0+j, 2+j, 4+j, 6+j}. Per-half order ig -> cg -> fg -> og.
                # Each chunk: prologue (gx via identity-matmul into PSUM + first
                # half of the contraction) then completion (second half),
                # staggered so chunk completions -- and the ACT drains that read
                # PSUM directly -- spread across the step.
                ALLCH = [0, 4, 2, 6, 1, 5, 3, 7]
                with tc.tile_pool(name="p2_ps", bufs=8, space="PSUM") as p2_ps:
                    for t in range(rec_steps):
                        with nc.named_scope("step"):
                            gx_sb = p2_gx.tile([BS, G], F16, tag="gx_sb")
                            nc.gpsimd.dma_start(out=gx_sb,
                                                in_=gx[t % t_steps, :, :])

                            ps_t = {}
                            pend = list(ALLCH)

                            def chunk_uses_ident(n):
                                # og chunks drain via ACT-direct (identity-matmul
                                # injects gx in PSUM) to shorten the critical
                                # tail; others use DVE stt drains.
                                return n in (6, 7)

                            def wh_mms(n, lo, hi, first):
                                # contraction segment [lo, hi) in k-pair units
                                # (fp8 DoubleRow) or k units (fp16)
                                col = slice(512 * n, 512 * (n + 1))
                                if wh_fp8:
                                    for i in range(lo // 2, hi // 2):
                                        nc.tensor.matmul(
                                            ps_t[n], lhsT=hTp[i],
                                            rhs=WhT_sb[:, 2 * i:2 * i + 2, col],
                                            perf_mode=DR,
                                            start=(first and i == lo // 2),
                                            stop=(hi == KH and i == KH // 2 - 1))
                                else:
                                    for k in range(lo, hi):
                                        nc.tensor.matmul(
                                            ps_t[n], lhsT=hTp[k],
                                            rhs=WhT_sb[:, k, col],
                                            start=(first and k == lo),
                                            stop=(k == KH - 1))

                            def prologue(n):
                                ps_t[n] = p2_ps.tile([BS, 512], F32, tag="ps2",
                                                     name="ps2")
                                if chunk_uses_ident(n):
                                    nc.tensor.matmul(
                                        ps_t[n], lhsT=ident_sb,
                                        rhs=gx_sb[:, 512 * n:512 * (n + 1)],
                                        start=True, stop=False)
                                wh_mms(n, 0, 4, first=not chunk_uses_ident(n))

                            def finish_chunk(n):
                                wh_mms(n, 4, KH, first=False)
                                if pend:
                                    prologue(pend.pop(0))
                                func = AFT.Tanh if n in (4, 5) else AFT.Sigmoid
                                ga = p2_act.tile([BS, 512], F16, tag="ga",
                                                 name="ga")
                                if chunk_uses_ident(n):
                                    nc.scalar.activation(out=ga, in_=ps_t[n],
                                                         func=func)
                                else:
                                    gs = p2_act.tile([BS, 512], F16, tag="gs",
                                                     name="gs")
                                    nc.vector.scalar_tensor_tensor(
                                        out=gs, in0=ps_t[n], scalar=1.0,
                                        in1=gx_sb[:, 512 * n:512 * (n + 1)],
                                        op0=AOP.mult, op1=AOP.add)
                                    nc.scalar.activation(out=ga, in_=gs,
                                                         func=func)
                                return ga

                            for _ in range(3):
                                prologue(pend.pop(0))

                            hT16_new, hTp_new = [], []
                            for j in range(2):    # unit halves 0:512, 512:1024
                                ch = c_st[:, 512 * j:512 * (j + 1)]
                                ig_a = finish_chunk(0 + j)
                                tcg = finish_chunk(4 + j)
                                t2 = p2_cell.tile([BS, 512], F16, tag="t2")
                                nc.vector.tensor_tensor(out=t2, in0=ig_a,
                                                        in1=tcg, op=AOP.mult)
                                fg_a = finish_chunk(2 + j)
                                t1 = p2_cell.tile([BS, 512], F32, tag="t1")
                                nc.vector.tensor_tensor(out=t1, in0=fg_a,
                                                        in1=ch, op=AOP.mult)
                                nc.vector.tensor_tensor(out=ch, in0=t1, in1=t2,
                                                        op=AOP.add)
                                tc_t = p2_cell.tile([BS, 512], F16, tag="tc_t")
                                nc.scalar.activation(out=tc_t, in_=ch,
                                                     func=AFT.Tanh)
                                og_a = finish_chunk(6 + j)
                                h_half = p2_cell.tile([BS, 512], F16,
                                                      tag="h_half")
                                nc.vector.tensor_tensor(out=h_half, in0=og_a,
                                                        in1=tc_t, op=AOP.mult)
                                # one xbar transpose: [32,512] -> [128,4,32] with
                                # [:, q, :] = h_T[512j+128q : 512j+128(q+1), :]
                                htn = p2_ht.tile([P, 4, BS], F16, tag="ht",
                                                 name="htn")
                                nc.sync.dma_start_transpose(out=htn, in_=h_half)
                                hT16_new.extend(htn[:, q, :] for q in range(4))
                                if wh_fp8:
                                    ht8 = p2_ht8.tile([P, 4, BS], F8, tag="ht8",
                                                      name="ht8n")
                                    nc.scalar.activation(out=ht8, in_=htn,
                                                         func=AFT.Copy)
                                    hTp_new.extend([ht8[:, 0:2, :],
                                                    ht8[:, 2:4, :]])
                                else:
                                    hTp_new.extend(htn[:, q, :]
                                                   for q in range(4))
                            hT16, hTp = hT16_new, hTp_new

                # ------- head -------
                with nc.named_scope("head"):
                    with tc.tile_pool(name="head_ps", bufs=1,
                                      space="PSUM") as hps:
                        ps_f = hps.tile([BS, F], F32, tag="ps_f")
                        for k in range(KH):
                            nc.tensor.matmul(ps_f, lhsT=hT16[k],
                                             rhs=WfaT_sb[:, k, :],
                                             start=(k == 0), stop=(k == KH - 1))
                        x1 = p2_cell.tile([BS, F], F32, tag="x1", bufs=1)
                        nc.vector.scalar_tensor_tensor(
                            out=x1, in0=ps_f, scalar=1.0, in1=bfa_rep,
                            op0=AOP.mult, op1=AOP.add)
                        x1r = p2_cell.tile([BS, F], F16, tag="x1r", bufs=1)
                        nc.scalar.activation(out=x1r, in_=x1, func=AFT.Relu)
                        x1T = p2_ht.tile([P, F // P, BS], F16, tag="x1T",
                                         bufs=1)
                        nc.sync.dma_start_transpose(out=x1T, in_=x1r)
                        ps_o = hps.tile([BS, NCLS], F32, tag="ps_o")
                        for q in range(F // P):
                            nc.tensor.matmul(ps_o, lhsT=x1T[:, q, :],
                                             rhs=WfcT_sb[:, q, :],
                                             start=(q == 0),
                                             stop=(q == F // P - 1))
                        out_sb = p2_cell.tile([BS, NCLS], F32, tag="out_sb",
                                              bufs=1)
                        nc.vector.scalar_tensor_tensor(
                            out=out_sb, in0=ps_o, scalar=1.0, in1=bfc_rep,
                            op0=AOP.mult, op1=AOP.add)
                        nc.sync.dma_start(out=out[:], in_=out_sb)

    nc.compile()
    return nc


def make_in_maps(inputs: dict, t_steps: int = T, wh_fp8: bool = WH_FP8):
    """Shard + lay out the full inputs for the 8 cores (host-side numpy)."""
    x = np.asarray(inputs["i_features"], np.float32)[:, :t_steps, :]
    v = np.asarray(inputs["v_features"], np.float32)
    Wi, bi = np.asarray(inputs["Wi"], np.float32), np.asarray(inputs["bi"], np.float32)
    Wh, bh = np.asarray(inputs["Wh"], np.float32), np.asarray(inputs["bh"], np.float32)
    Wc, bc = np.asarray(inputs["Wc"], np.float32), np.asarray(inputs["bc"], np.float32)
    Wfa, bfa = np.asarray(inputs["Wfa"], np.float32), np.asarray(inputs["bfa"], np.float32)
    Wfc, bfc = np.asarray(inputs["Wfc"], np.float32), np.asarray(inputs["bfc"], np.float32)

    whdt = mybir.dt.np(F8) if wh_fp8 else np.float16
    shared = {
        "WiT": np.ascontiguousarray(Wi.T).astype(np.float16),
        "WhT": np.ascontiguousarray(Wh.T).astype(whdt),
        "WcT": np.ascontiguousarray(Wc.T).astype(np.float16),
        "WfaT": np.ascontiguousarray(Wfa.T).astype(np.float16),
        "WfcT": np.ascontiguousarray(Wfc.T).astype(np.float16),
        "bias_total": (bi + bh + bc).astype(np.float32),
        "bfa": bfa.astype(np.float32),
        "bfc": bfc.astype(np.float32),
        "ident32": np.eye(BS, dtype=np.float16),
    }
    in_maps = []
    nb = x.shape[0] // BS
    for s in range(nb):
        xs = x[s * BS:(s + 1) * BS]                      # [BS, t, I]
        in_maps.append({
            "xT": np.ascontiguousarray(xs.transpose(2, 1, 0)).astype(np.float16),
            "vT": np.ascontiguousarray(v[s * BS:(s + 1) * BS].T).astype(np.float16),
            **shared,
        })
    return in_maps


_NC_CACHE = {}


def finish_output(per_core_outs: list) -> np.ndarray:
    """Host-side gather of per-core 'out' tensors into the full [B, NC] result."""
    return np.concatenate(per_core_outs, axis=0).astype(np.float32)


def kernel(**inputs) -> np.ndarray:
    in_maps = make_in_maps(inputs, T)
    if T not in _NC_CACHE:
        _NC_CACHE[T] = build_lstm(T)
    nc = _NC_CACHE[T]
    res = run_bass_kernel_spmd(nc, in_maps, core_ids=list(range(NCORES)))
    return finish_output([r["out"] for r in res.results])


# revision 9
# speedup vs baseline: 14.8145x; 1.0668x over previous
"""Trainium2 Bass kernel for a context-LSTM decoder.

Model (B=256, T=256, I=H=1024, 4H=4096, F=512, NC=7):
    ctx   = v @ Wc.T + (bc + bi + bh)                      # [B, 4H], const over t
    gx    = i_features @ Wi.T + ctx                        # [B, T, 4H]
    per t: gates = gx[t] + h @ Wh.T ; LSTM cell update
    out   = relu(h_T @ Wfa.T + bfa) @ Wfc.T + bfc          # [B, 7]

Strategy: pure data-parallel over batch, 32 rows per core, no collectives.
All matmul operands fp16 (fp32 PSUM accumulation), cell state c fp32.
Host pre-transposes x to [I, T, Bs] and weights to K-major so every DMA is
contiguous; the only on-device transposes are the per-step h -> h_T
(DMA xbar transpose, off the critical PE path).

Phase 1 (big GEMM) tiles M=(4t x 32b)=128 rows at full PE width; phase 2
streams Wh through the PE as the moving operand (64 matmuls of N=512 per
step, ~13.2us/step measured = the fp16 streaming roofline); DVE adds gx
during PSUM drain and ACT does sigmoid/tanh.  An optional fp8e4+DoubleRow
path for h@Wh.T exists (WH_FP8) but measured slower than fp16 -- see flag
comment.
"""

import numpy as np
from contextlib import ExitStack

import concourse.bass as bass
import concourse.bacc as bacc
import concourse.mybir as mybir
import concourse.tile as tile
from concourse.bass_utils import run_bass_kernel_spmd

B, T, I = 256, 256, 1024
H = 1024
G = 4 * H
F = 512
NCLS = 7
NCORES = 8
BS = B // NCORES          # 32 batch rows per core
P = 128                   # partitions
KI = I // P               # 8 k-tiles over the input dim
KH = H // P               # 8 k-tiles over the hidden dim
NCH = G // 512            # 8 chunks of 512 gate columns
F16 = mybir.dt.float16
F32 = mybir.dt.float32
F8 = mybir.dt.float8e4
AOP = mybir.AluOpType
AFT = mybir.ActivationFunctionType
DR = mybir.MatmulPerfMode.DoubleRow

# fp8e4 + DoubleRow for the recurrence h @ Wh.T.  Measured on hardware it is
# a net LOSS (15.9us/step vs 13.2us/step fp16; DoubleRow's weight-load penalty
# and the extra fp16->fp8 converts outweigh the 2-rows/cycle streaming), and
# it costs 12x in accuracy (rel err 8e-3 vs 6.4e-4) -- so it stays off.
WH_FP8 = False


def build_lstm(t_steps: int = T, rec_steps: int | None = None, reps: int = 1,
               wh_fp8: bool = WH_FP8):
    # rec_steps: number of recurrence iterations (defaults to t_steps). When
    # larger than t_steps, gx rows are reused cyclically (timing studies only).
    # reps: repeat the whole computation in-NEFF (timing studies only).
    if rec_steps is None:
        rec_steps = t_steps
    assert t_steps % 4 == 0
    n_mtiles = t_steps * BS // P          # phase-1 M-tiles (128 bt-rows each)
    nc = bacc.Bacc("TRN2", target_bir_lowering=False, debug=False,
                   num_devices=NCORES)
    WHDT = F8 if wh_fp8 else F16

    xT = nc.declare_dram_parameter("xT", [I, t_steps, BS], F16, isOutput=False)
    vT = nc.declare_dram_parameter("vT", [I, BS], F16, isOutput=False)
    WiT = nc.declare_dram_parameter("WiT", [I, G], F16, isOutput=False)
    WhT = nc.declare_dram_parameter("WhT", [H, G], WHDT, isOutput=False)
    WcT = nc.declare_dram_parameter("WcT", [I, G], F16, isOutput=False)
    WfaT = nc.declare_dram_parameter("WfaT", [H, F], F16, isOutput=False)
    WfcT = nc.declare_dram_parameter("WfcT", [F, NCLS], F16, isOutput=False)
    bias = nc.declare_dram_parameter("bias_total", [G], F32, isOutput=False)
    bfa = nc.declare_dram_parameter("bfa", [F], F32, isOutput=False)
    bfc = nc.declare_dram_parameter("bfc", [NCLS], F32, isOutput=False)
    ident = nc.declare_dram_parameter("ident32", [BS, BS], F16, isOutput=False)
    out = nc.declare_dram_parameter("out", [BS, NCLS], F32, isOutput=True)

    gx = nc.dram_tensor("gx", [t_steps, BS, G], F16)

    # K-major views of DRAM tensors: i = k*128 + p
    xT_r = xT[:].rearrange("(k p) t b -> p k (t b)", p=P)
    WiT_r = WiT[:].rearrange("(k p) g -> p k g", p=P)
    WhT_r = WhT[:].rearrange("(k p) g -> p k g", p=P)
    WcT_r = WcT[:].rearrange("(k p) g -> p k g", p=P)
    WfaT_r = WfaT[:].rearrange("(k p) f -> p k f", p=P)
    WfcT_r = WfcT[:].rearrange("(q p) n -> p q n", p=P)
    vT_r = vT[:].rearrange("(k p) b -> p k b", p=P)
    gx_rows = gx[:].rearrange("t b g -> (t b) g")

    def bcast(src_ap, rows):
        # read a [cols] DRAM vector into [rows, cols] SBUF (partition bcast)
        return bass.AP(tensor=src_ap.tensor, offset=src_ap.offset,
                       ap=[[0, rows]] + list(src_ap.ap))

    with tile.TileContext(nc) as tc, ExitStack() as ctx:
        consts = ctx.enter_context(tc.tile_pool(name="consts", bufs=1))

        # ---- small resident constants ----
        bfa_rep = consts.tile([BS, F], F32, tag="bfa_rep")
        nc.sync.dma_start(out=bfa_rep, in_=bcast(bfa[:], BS))
        bfc_rep = consts.tile([BS, NCLS], F32, tag="bfc_rep")
        nc.sync.dma_start(out=bfc_rep, in_=bcast(bfc[:], BS))
        ident_sb = consts.tile([BS, BS], F16, tag="ident_sb")
        nc.sync.dma_start(out=ident_sb, in_=ident[:])

        for _rep in range(reps):
            with (
                tc.tile_pool(name="ph1_big", bufs=1) as ph1_big,
                tc.tile_pool(name="ph1_ps", bufs=6, space="PSUM") as ph1_ps,
                tc.tile_pool(name="ph1_x", bufs=3) as ph1_x,
                tc.tile_pool(name="ph1_out", bufs=4) as ph1_out,
            ):
                ctx4 = ph1_big.tile([P, G], F32, tag="ctx4")
                # ------- phase 0: ctx4 = v @ WcT + biases, replicated 4x -------
                with nc.named_scope("phase0_ctx"):
                    with (
                        tc.tile_pool(name="ph0_w", bufs=2) as ph0_w,
                        tc.tile_pool(name="ph0_misc", bufs=1) as ph0_misc,
                    ):
                        v_sb = ph0_misc.tile([P, KI, BS], F16, tag="v_sb")
                        nc.sync.dma_start(out=v_sb, in_=vT_r)
                        bias_rep = ph0_misc.tile([BS, G], F32, tag="bias_rep")
                        nc.sync.dma_start(out=bias_rep, in_=bcast(bias[:], BS))
                        for n in range(NCH):
                            wc_t = ph0_w.tile([P, KI, 512], F16, tag="wc")
                            nc.sync.dma_start(
                                out=wc_t, in_=WcT_r[:, :, 512 * n:512 * (n + 1)])
                            ps0 = ph1_ps.tile([BS, 512], F32, tag="ps0", bufs=2)
                            for k in range(KI):
                                nc.tensor.matmul(ps0, lhsT=v_sb[:, k, :],
                                                 rhs=wc_t[:, k, :],
                                                 start=(k == 0),
                                                 stop=(k == KI - 1))
                            nc.vector.scalar_tensor_tensor(
                                out=ctx4[0:BS, 512 * n:512 * (n + 1)], in0=ps0,
                                scalar=1.0,
                                in1=bias_rep[:, 512 * n:512 * (n + 1)],
                                op0=AOP.mult, op1=AOP.add)
                        for r in range(1, 4):
                            nc.sync.dma_start(out=ctx4[BS * r:BS * (r + 1), :],
                                              in_=ctx4[0:BS, :])

                # ------- phase 1: gx = x @ WiT + ctx4 -------
                with nc.named_scope("phase1_gemm"):
                    WiT_sb = ph1_big.tile([P, KI, G], F16, tag="WiT_sb")
                    for k in range(KI):
                        nc.sync.dma_start(out=WiT_sb[:, k, :], in_=WiT_r[:, k, :])

                    n_groups = (n_mtiles + 3) // 4
                    for grp in range(n_groups):
                        mt_in_grp = min(4, n_mtiles - grp * 4)
                        x_sb = ph1_x.tile([P, KI, 512], F16, tag="x_sb")
                        nc.sync.dma_start(
                            out=x_sb[:, :, 0:128 * mt_in_grp],
                            in_=xT_r[:, :, 512 * grp:512 * grp + 128 * mt_in_grp])
                        for mi in range(mt_in_grp):
                            m = grp * 4 + mi
                            # batch the whole m-tile's output so the gx write is
                            # one fully-contiguous 1MB transfer (rows of [T,BS,G]
                            # are contiguous) instead of 8 strided 1KB-line DMAs.
                            gxo = ph1_out.tile([P, G], F16, tag="gxo")
                            for n in range(NCH):
                                ps1 = ph1_ps.tile([P, 512], F32, tag="ps1")
                                for k in range(KI):
                                    nc.tensor.matmul(
                                        ps1,
                                        lhsT=x_sb[:, k, 128 * mi:128 * (mi + 1)],
                                        rhs=WiT_sb[:, k, 512 * n:512 * (n + 1)],
                                        start=(k == 0), stop=(k == KI - 1))
                                nc.vector.scalar_tensor_tensor(
                                    out=gxo[:, 512 * n:512 * (n + 1)], in0=ps1,
                                    scalar=1.0,
                                    in1=ctx4[:, 512 * n:512 * (n + 1)],
                                    op0=AOP.mult, op1=AOP.add)
                            nc.gpsimd.dma_start(
                                out=gx_rows[128 * m:128 * (m + 1), :], in_=gxo)

            # ------- phase 2: recurrence -------
            with (
                tc.tile_pool(name="p2_w", bufs=1) as p2_w,
                tc.tile_pool(name="p2_state", bufs=1) as p2_state,
                tc.tile_pool(name="p2_gx", bufs=6) as p2_gx,
                tc.tile_pool(name="p2_act", bufs=10) as p2_act,
                tc.tile_pool(name="p2_cell", bufs=4) as p2_cell,
                tc.tile_pool(name="p2_ht", bufs=12) as p2_ht,
                tc.tile_pool(name="p2_ht8", bufs=12) as p2_ht8,
            ):
                WhT_sb = p2_w.tile([P, KH, G], WHDT, tag="WhT_sb")
                for k in range(KH):
                    nc.sync.dma_start(out=WhT_sb[:, k, :], in_=WhT_r[:, k, :])
                WfaT_sb = p2_w.tile([P, KH, F], F16, tag="WfaT_sb")
                nc.sync.dma_start(out=WfaT_sb, in_=WfaT_r)
                WfcT_sb = p2_w.tile([P, F // P, NCLS], F16, tag="WfcT_sb")
                nc.sync.dma_start(out=WfcT_sb, in_=WfcT_r)

                c_st = p2_state.tile([BS, H], F32, tag="c_st")        # [32, 1024]
                nc.vector.memset(c_st, 0.0)
                # hT16: fp16 transposed h (also feeds the head); hTp: what the
                # Wh matmuls consume -- fp8 pair-tiles (DoubleRow) or the fp16
                # slices themselves.
                hT16 = []
                hTp = []
                for j in range(2):
                    ht0 = p2_ht.tile([P, 4, BS], F16, tag="ht", name="ht0")
                    nc.vector.memset(ht0, 0.0)
                    hT16.extend(ht0[:, q, :] for q in range(4))
                    if wh_fp8:
                        ht8 = p2_ht8.tile([P, 4, BS], F8, tag="ht8", name="ht80")
                        nc.vector.memset(ht8, 0.0)
                        hTp.extend([ht8[:, 0:2, :], ht8[:, 2:4, :]])
                    else:
                        hTp.extend([ht0[:, q, :] for q in range(4)])

                # chunk semantics: ig=0,1 fg=2,3 cg=4,5 og=6,7; half j uses
                # chunks {0+j, 2+j, 4+j, 6+j}. Per-half order ig -> cg -> fg -> og.
                # Each chunk: prologue (gx via identity-matmul into PSUM + first
                # half of the contraction) then completion (second half),
                # staggered so chunk completions -- and the ACT drains that read
                # PSUM directly -- spread across the step.
                ALLCH = [0, 4, 2, 6, 1, 5, 3, 7]
                with tc.tile_pool(name="p2_ps", bufs=8, space="PSUM") as p2_ps:
                    for t in range(rec_steps):
                        with nc.named_scope("step"):
                            gx_sb = p2_gx.tile([BS, G], F16, tag="gx_sb")
                            nc.gpsimd.dma_start(out=gx_sb,
                                                in_=gx[t % t_steps, :, :])

                            ps_t = {}
                            pend = list(ALLCH)

                            def chunk_uses_ident(n):
                                # og chunks drain via ACT-direct (identity-matmul
                                # injects gx in PSUM) to shorten the critical
                                # tail; others use DVE stt drains.
                                return n in (6, 7)

                            def wh_mms(n, lo, hi, first):
                                # contraction segment [lo, hi) in k-pair units
                                # (fp8 DoubleRow) or k units (fp16)
                                col = slice(512 * n, 512 * (n + 1))
                                if wh_fp8:
                                    for i in range(lo // 2, hi // 2):
                                        nc.tensor.matmul(
                                            ps_t[n], lhsT=hTp[i],
                                            rhs=WhT_sb[:, 2 * i:2 * i + 2, col],
                                            perf_mode=DR,
                                            start=(first and i == lo // 2),
                                            stop=(hi == KH and i == KH // 2 - 1))
                                else:
                                    for k in range(lo, hi):
                                        nc.tensor.matmul(
                                            ps_t[n], lhsT=hTp[k],
                                            rhs=WhT_sb[:, k, col],
                                            start=(first and k == lo),
                                            stop=(k == KH - 1))

                            def prologue(n):
                                ps_t[n] = p2_ps.tile([BS, 512], F32, tag="ps2",
                                                     name="ps2")
                                if chunk_uses_ident(n):
                                    nc.tensor.matmul(
                                        ps_t[n], lhsT=ident_sb,
                                        rhs=gx_sb[:, 512 * n:512 * (n + 1)],
                                        start=True, stop=False)
                                wh_mms(n, 0, 4, first=not chunk_uses_ident(n))

                            def finish_chunk(n):
                                wh_mms(n, 4, KH, first=False)
                                if pend:
                                    prologue(pend.pop(0))
                                func = AFT.Tanh if n in (4, 5) else AFT.Sigmoid
                                ga = p2_act.tile([BS, 512], F16, tag="ga",
                                                 name="ga")
                                if chunk_uses_ident(n):
                                    nc.scalar.activation(out=ga, in_=ps_t[n],
                                                         func=func)
                                else:
                                    gs = p2_act.tile([BS, 512], F16, tag="gs",
                                                     name="gs")
                                    nc.vector.scalar_tensor_tensor(
                                        out=gs, in0=ps_t[n], scalar=1.0,
                                        in1=gx_sb[:, 512 * n:512 * (n + 1)],
                                        op0=AOP.mult, op1=AOP.add)
                                    nc.scalar.activation(out=ga, in_=gs,
                                                         func=func)
                                return ga

                            for _ in range(3):
                                prologue(pend.pop(0))

                            hT16_new, hTp_new = [], []
                            for j in range(2):    # unit halves 0:512, 512:1024
                                ch = c_st[:, 512 * j:512 * (j + 1)]
                                ig_a = finish_chunk(0 + j)
                                tcg = finish_chunk(4 + j)
                                t2 = p2_cell.tile([BS, 512], F16, tag="t2")
                                nc.vector.tensor_tensor(out=t2, in0=ig_a,
                                                        in1=tcg, op=AOP.mult)
                                fg_a = finish_chunk(2 + j)
                                t1 = p2_cell.tile([BS, 512], F32, tag="t1")
                                nc.vector.tensor_tensor(out=t1, in0=fg_a,
                                                        in1=ch, op=AOP.mult)
                                nc.vector.tensor_tensor(out=ch, in0=t1, in1=t2,
                                                        op=AOP.add)
                                tc_t = p2_cell.tile([BS, 512], F16, tag="tc_t")
                                nc.scalar.activation(out=tc_t, in_=ch,
                                                     func=AFT.Tanh)
                                og_a = finish_chunk(6 + j)
                                h_half = p2_cell.tile([BS, 512], F16,
                                                      tag="h_half")
                                nc.vector.tensor_tensor(out=h_half, in0=og_a,
                                                        in1=tc_t, op=AOP.mult)
                                # one xbar transpose: [32,512] -> [128,4,32] with
                                # [:, q, :] = h_T[512j+128q : 512j+128(q+1), :]
                                htn = p2_ht.tile([P, 4, BS], F16, tag="ht",
                                                 name="htn")
                                nc.sync.dma_start_transpose(out=htn, in_=h_half)
                                hT16_new.extend(htn[:, q, :] for q in range(4))
                                if wh_fp8:
                                    ht8 = p2_ht8.tile([P, 4, BS], F8, tag="ht8",
                                                      name="ht8n")
                                    nc.scalar.activation(out=ht8, in_=htn,
                                                         func=AFT.Copy)
                                    hTp_new.extend([ht8[:, 0:2, :],
                                                    ht8[:, 2:4, :]])
                                else:
                                    hTp_new.extend(htn[:, q, :]
                                                   for q in range(4))
                            hT16, hTp = hT16_new, hTp_new

                # ------- head -------
                with nc.named_scope("head"):
                    with tc.tile_pool(name="head_ps", bufs=1,
                                      space="PSUM") as hps:
                        ps_f = hps.tile([BS, F], F32, tag="ps_f")
                        for k in range(KH):
                            nc.tensor.matmul(ps_f, lhsT=hT16[k],
                                             rhs=WfaT_sb[:, k, :],
                                             start=(k == 0), stop=(k == KH - 1))
                        x1 = p2_cell.tile([BS, F], F32, tag="x1", bufs=1)
                        nc.vector.scalar_tensor_tensor(
                            out=x1, in0=ps_f, scalar=1.0, in1=bfa_rep,
                            op0=AOP.mult, op1=AOP.add)
                        x1r = p2_cell.tile([BS, F], F16, tag="x1r", bufs=1)
                        nc.scalar.activation(out=x1r, in_=x1, func=AFT.Relu)
                        x1T = p2_ht.tile([P, F // P, BS], F16, tag="x1T",
                                         bufs=1)
                        nc.sync.dma_start_transpose(out=x1T, in_=x1r)
                        ps_o = hps.tile([BS, NCLS], F32, tag="ps_o")
                        for q in range(F // P):
                            nc.tensor.matmul(ps_o, lhsT=x1T[:, q, :],
                                             rhs=WfcT_sb[:, q, :],
                                             start=(q == 0),
                                             stop=(q == F // P - 1))
                        out_sb = p2_cell.tile([BS, NCLS], F32, tag="out_sb",
                                              bufs=1)
                        nc.vector.scalar_tensor_tensor(
                            out=out_sb, in0=ps_o, scalar=1.0, in1=bfc_rep,
                            op0=AOP.mult, op1=AOP.add)
                        nc.sync.dma_start(out=out[:], in_=out_sb)

    nc.compile()
    return nc


def make_in_maps(inputs: dict, t_steps: int = T, wh_fp8: bool = WH_FP8):
    """Shard + lay out the full inputs for the 8 cores (host-side numpy)."""
    x = np.asarray(inputs["i_features"], np.float32)[:, :t_steps, :]
    v = np.asarray(inputs["v_features"], np.float32)
    Wi, bi = np.asarray(inputs["Wi"], np.float32), np.asarray(inputs["bi"], np.float32)
    Wh, bh = np.asarray(inputs["Wh"], np.float32), np.asarray(inputs["bh"], np.float32)
    Wc, bc = np.asarray(inputs["Wc"], np.float32), np.asarray(inputs["bc"], np.float32)
    Wfa, bfa = np.asarray(inputs["Wfa"], np.float32), np.asarray(inputs["bfa"], np.float32)
    Wfc, bfc = np.asarray(inputs["Wfc"], np.float32), np.asarray(inputs["bfc"], np.float32)

    whdt = mybir.dt.np(F8) if wh_fp8 else np.float16
    shared = {
        "WiT": np.ascontiguousarray(Wi.T).astype(np.float16),
        "WhT": np.ascontiguousarray(Wh.T).astype(whdt),
        "WcT": np.ascontiguousarray(Wc.T).astype(np.float16),
        "WfaT": np.ascontiguousarray(Wfa.T).astype(np.float16),
        "WfcT": np.ascontiguousarray(Wfc.T).astype(np.float16),
        "bias_total": (bi + bh + bc).astype(np.float32),
        "bfa": bfa.astype(np.float32),
        "bfc": bfc.astype(np.float32),
        "ident32": np.eye(BS, dtype=np.float16),
    }
    in_maps = []
    nb = x.shape[0] // BS
    for s in range(nb):
        xs = x[s * BS:(s + 1) * BS]                      # [BS, t, I]
        in_maps.append({
            "xT": np.ascontiguousarray(xs.transpose(2, 1, 0)).astype(np.float16),
            "vT": np.ascontiguousarray(v[s * BS:(s + 1) * BS].T).astype(np.float16),
            **shared,
        })
    return in_maps


_NC_CACHE = {}


def finish_output(per_core_outs: list) -> np.ndarray:
    """Host-side gather of per-core 'out' tensors into the full [B, NC] result."""
    return np.concatenate(per_core_outs, axis=0).astype(np.float32)


def kernel(**inputs) -> np.ndarray:
    in_maps = make_in_maps(inputs, T)
    if T not in _NC_CACHE:
        _NC_CACHE[T] = build_lstm(T)
    nc = _NC_CACHE[T]
    res = run_bass_kernel_spmd(nc, in_maps, core_ids=list(range(NCORES)))
    return finish_output([r["out"] for r in res.results])
